# revision 15
# baseline (speedup 1.0000x reference)
"""Trainium2 Bass kernel for the 2-layer GraphSAGE bus-stop predictor.

Self-contained: kernel(**inputs) -> np.ndarray [N, 1].

Strategy (8 NeuronCores, SPMD, dst-sharded nodes):

L1 aggregation (x is only 4-wide):
  - x packed bf16, 32 nodes per 256B gather element -> the whole 1M-node
    table is 31250 elements = ONE int16 window (8MB).
  - Per-core nodes sorted into rectangular [128 x d] degree-class groups
    (d in {1,2,4,8,16,32}); gather calls stream group columns; an on-chip
    one-hot (iota==v) select + fused reduce sums each group's columns
    straight into an SBUF-resident agg1 accumulator. NO scatter-add.
L2 aggregation (h1 is 64-wide):
  - h1 stored bf16 paired (2 nodes per 256B element); chunked AllGather
    builds the global table (16 windows x 4 dst-quarters packing cells).
  - gather pair -> on-chip mask blend extracts the right half ->
    dma_scatter_add (CCE) into fp32 agg2 in HBM.
Dense phases: BN folded into weights; L1 uses a combined [x|agg] K=16
matmul; L2 keeps h1 feature-major in DRAM so the root term needs no
transpose; the 1-wide head + sigmoid is fused into the L2 loop.
"""

import os
import time

import numpy as np
import ml_dtypes

import concourse.bacc as bacc
import concourse.mybir as mybir
import concourse.tile as tile
from concourse.bass_utils import run_bass_kernel_spmd

f32 = mybir.dt.float32
bf16 = mybir.dt.bfloat16
i16 = mybir.dt.int16
BF = ml_dtypes.bfloat16

AF = mybir.ActivationFunctionType
OP = mybir.AluOpType

N = 1_000_000
C = 8                    # cores
SLOC = N // C            # nodes per core
PACK1 = 32               # x nodes per 256B gather element
XEL = N // PACK1         # 31250 L1 table elements (single int16 window)
# SWDGE call sizes. The IndirectLoad completion-semaphore wait is
# num_idxs * elem_bytes/4 (+4) and must fit 16 bits -> 1024x256B is
# exactly over. L1 keeps full 128-row columns (7 per call); L2 is flat.
CALL1 = 896
COLS1 = 7
CALL2 = 960
WIN2 = 32768             # L2 gather window (elements)
CHUNK = 512              # dense-phase nodes per chunk
CLS = (1, 2, 4, 8, 16, 32)   # padded degree classes (0 handled separately)
PADV = 200.0             # one-hot sentinel: matches no iota value
EPS = 1e-5

LAST_EXEC_NS = None
LAST_RESULTS = None


# ---------------------------------------------------------------- host prep

def _wrap_idx(vals, call):
    """int16 vals [k, call] -> dma idx tiles [k, 128, call//16]."""
    m = np.asarray(vals, dtype=np.int16).reshape(-1, call)
    k = m.shape[0]
    out = np.zeros((k, 128, call // 16), dtype=np.int16)
    ii = np.arange(call)
    for g in range(8):
        out[:, (ii % 16) + 16 * g, ii // 16] = m
    return out


def _agg2_row(pos):
    """Position -> agg2 row (in-chunk interleave so dense loads have
    1KB-contiguous runs per partition)."""
    pm = pos % CHUNK
    return (pos // CHUNK) * CHUNK + (pm % 128) * 4 + pm // 128


def _host_prep(src, dst):
    deg = np.bincount(dst, minlength=N).astype(np.int64)
    inv = (1.0 / np.maximum(deg, 1.0)).astype(np.float32)
    assert int(deg.max()) <= CLS[-1], int(deg.max())

    degc = deg.reshape(C, SLOC)
    pcls = np.zeros((C, SLOC), np.int64)
    lo = 0
    for k in CLS:
        pcls[(degc > lo) & (degc <= k)] = k
        lo = k

    cnt = {k: np.array([(pcls[c] == k).sum() for c in range(C)]) for k in CLS}
    G = {k: int(np.ceil((cnt[k].max() + 4) / 128)) for k in CLS}
    rows_used = 128 * sum(G.values())
    cnt0 = np.array([(pcls[c] == 0).sum() for c in range(C)])
    rows0 = int(np.ceil((cnt0.max() + 4) / 128)) * 128
    S = rows_used + rows0
    S = ((S + 2047) // 2048) * 2048
    q = S // 4
    assert q <= 32767, S
    reserved = np.array([q - 1, 2 * q - 1, 3 * q - 1, 4 * q - 1])
    GG = S // 128
    classbase = {}
    b = 0
    for k in CLS:
        classbase[k] = b
        b += 128 * G[k]
    zerobase = b

    # per-core incoming-edge CSR (by dst) and node positions
    pos_of_local = []
    core_csr = []
    for c in range(C):
        m = (dst // SLOC) == c
        sd = src[m]
        dl = dst[m] - c * SLOC
        o = np.argsort(dl, kind="stable")
        sd = sd[o]
        starts = np.searchsorted(dl[o], np.arange(SLOC + 1))
        core_csr.append((sd, starts))
        pos = np.full(SLOC, -1, np.int64)
        for k in CLS + (0,):
            nodes = np.nonzero(pcls[c] == k)[0]
            base = classbase[k] if k else zerobase
            length = 128 * G[k] if k else S - zerobase
            pr = base + np.arange(length)
            pr = pr[~np.isin(pr, reserved)][: len(nodes)]
            assert len(pr) == len(nodes), (c, k, len(nodes))
            pos[nodes] = pr
        pos_of_local.append(pos)

    # ---- L1 gather streams + shared reduce-fragment schedule
    frags = {}           # call -> list[(c0, c1, gg, accum)]
    callbase = {}
    ncalls1 = 0
    for k in CLS:
        callbase[k] = ncalls1
        ncalls1 += (G[k] * k + COLS1 - 1) // COLS1
        for g in range(G[k]):
            gg = classbase[k] // 128 + g
            c0 = g * k
            first = True
            while c0 < (g + 1) * k:
                call = callbase[k] + c0 // COLS1
                e = min((g + 1) * k, (c0 // COLS1 + 1) * COLS1)
                frags.setdefault(call, []).append(
                    (c0 % COLS1, ((e - 1) % COLS1) + 1, gg, not first))
                first = False
                c0 = e

    idx1 = [[] for _ in range(C)]
    vr1 = [[] for _ in range(C)]
    for c in range(C):
        sd, starts = core_csr[c]
        lens = np.diff(starts)
        pos = pos_of_local[c]
        for k in CLS:
            nodes = np.nonzero(pcls[c] == k)[0]
            rows = np.full(G[k] * 128, -1, np.int64)
            rows[pos[nodes] - classbase[k]] = nodes
            srcmat = np.full((G[k] * 128, k), -1, np.int64)
            nz = np.nonzero(rows >= 0)[0]
            nd = rows[nz]
            if len(nd):
                ln = lens[nd]
                rr = np.repeat(nz, ln)
                cc2 = (np.arange(len(rr)) -
                       np.repeat(np.cumsum(
                           np.concatenate([[0], ln[:-1]])), ln))
                flat = np.repeat(starts[nd], ln) + cc2
                srcmat[rr, cc2] = sd[flat]
            st = srcmat.reshape(G[k], 128, k).transpose(0, 2, 1).reshape(-1)
            ncall_k = (G[k] * k + COLS1 - 1) // COLS1
            padn = ncall_k * CALL1 - len(st)
            st = np.concatenate([st, np.full(padn, -1, np.int64)])
            gi = np.where(st >= 0, st // PACK1, 0).astype(np.int16)
            vv = np.where(st >= 0, st % PACK1, PADV).astype(BF)
            idx1[c].append(gi.reshape(-1, CALL1))
            vr1[c].append(vv.reshape(-1, COLS1, 128).transpose(0, 2, 1))
        idx1[c] = _wrap_idx(np.concatenate(idx1[c]), CALL1)
        vr1[c] = np.concatenate(vr1[c]).astype(BF)
    assert idx1[0].shape[0] == ncalls1

    # ---- L2 cell streams
    T2 = C * S // 2
    nwin = (T2 + WIN2 - 1) // WIN2
    cells = [dict() for _ in range(C)]
    dump_row = _agg2_row(q - 1) % q
    for c in range(C):
        sd, starts = core_csr[c]
        lens = np.diff(starts)
        pos = pos_of_local[c]
        dposn = np.repeat(pos, lens)
        sc = sd // SLOC
        agq = S // 8           # elements per core per AllGather chunk
        elem = np.empty(len(sd), np.int64)
        half = np.empty(len(sd), np.int64)
        for cc in range(C):
            m2 = sc == cc
            pp = pos_of_local[cc][sd[m2] % SLOC]
            el = pp // 2
            # h1f is chunk-major: [chunk r][core cc][rows] so each chunked
            # AllGather output is contiguous
            elem[m2] = (el // agq) * (C * agq) + cc * agq + el % agq
            half[m2] = pp % 2
        w = elem // WIN2
        drow = _agg2_row(dposn)
        qq = drow // q
        sidx = drow % q
        order = np.lexsort((sidx, qq, w))
        elem, half, w, qq, sidx = (a[order] for a in (elem, half, w, qq, sidx))
        bounds = np.searchsorted(w * 4 + qq, np.arange(nwin * 4 + 1))
        for cell in range(nwin * 4):
            lo2, hi = bounds[cell], bounds[cell + 1]
            if lo2 == hi:
                continue
            cw, cq = cell // 4, cell % 4
            ge = (elem[lo2:hi] - cw * WIN2).astype(np.int16)
            hh = half[lo2:hi].astype(np.int16)
            ds = sidx[lo2:hi].astype(np.int16)
            calls = []
            pend = list(zip(ge.tolist(), hh.tolist(), ds.tolist()))
            while pend:
                cg, ch, cd = [], [], []
                seen = set()
                nxt = []
                for gg2, hh2, dd2 in pend:
                    if len(cg) < CALL2 and dd2 not in seen:
                        cg.append(gg2)
                        ch.append(hh2)
                        cd.append(dd2)
                        seen.add(dd2)
                    else:
                        nxt.append((gg2, hh2, dd2))
                while len(cg) < CALL2:
                    cg.append(0)
                    ch.append(-1)
                    cd.append(dump_row)
                calls.append((np.array(cg, np.int16), np.array(ch, np.int16),
                              np.array(cd, np.int16)))
                pend = nxt
            cells[c][(cw, cq)] = calls

    meta2 = []
    g2 = [[] for _ in range(C)]
    s2 = [[] for _ in range(C)]
    m2l = [[] for _ in range(C)]
    for cell in sorted({kk for cc in cells for kk in cc}):
        ncall = max(len(cc.get(cell, [])) for cc in cells)
        cw, cq = cell
        for kk in range(ncall):
            meta2.append((cw * WIN2, cq * q))
            for c in range(C):
                cl = cells[c].get(cell, [])
                if kk < len(cl):
                    cg, ch, cd = cl[kk]
                else:
                    cg = np.zeros(CALL2, np.int16)
                    ch = np.full(CALL2, -1, np.int16)
                    cd = np.full(CALL2, dump_row, np.int16)
                g2[c].append(cg)
                s2[c].append(cd)
                mm = np.zeros((1024, 2), BF)
                mm[:CALL2][ch == 0, 0] = 1
                mm[:CALL2][ch == 1, 1] = 1
                m2l[c].append(mm.reshape(8, 128, 2).transpose(1, 0, 2))
    ncalls2 = len(meta2)
    for c in range(C):
        g2[c] = _wrap_idx(np.concatenate(g2[c]), CALL2)
        s2[c] = _wrap_idx(np.concatenate(s2[c]), CALL2)
        m2l[c] = np.stack(m2l[c]).reshape(ncalls2, 128, 16)

    shared = dict(S=S, q=q, GG=GG, ncalls1=ncalls1, frags=frags,
                  meta2=meta2, ncalls2=ncalls2, T2=T2)
    payload = [dict(idx1=idx1[c], vr1=vr1[c], g2=g2[c], s2=s2[c], m2=m2l[c])
               for c in range(C)]
    return shared, payload, pos_of_local, inv


# ---------------------------------------------------------------- bass build

def _build(sh):
    S, q, GG = sh["S"], sh["q"], sh["GG"]
    ncalls1, ncalls2 = sh["ncalls1"], sh["ncalls2"]
    frags, meta2, T2 = sh["frags"], sh["meta2"], sh["T2"]
    nchunks = S // CHUNK
    NB1 = (ncalls1 + 7) // 8
    NB2 = (ncalls2 + 7) // 8

    nc = bacc.Bacc("TRN2", target_bir_lowering=False, debug=False,
                   num_devices=C)
    xtab = nc.dram_tensor("xtab", [XEL, 128], bf16, kind="ExternalInput")
    iotab = nc.dram_tensor("iotab", [128, 256], bf16, kind="ExternalInput")
    idx1 = nc.dram_tensor("idx1", [128, NB1 * 8, CALL1 // 16], i16,
                          kind="ExternalInput")
    vr1 = nc.dram_tensor("vr1", [128, NB1 * 8, COLS1], bf16,
                         kind="ExternalInput")
    g2d = nc.dram_tensor("g2", [128, NB2 * 8, CALL2 // 16], i16,
                         kind="ExternalInput")
    s2d = nc.dram_tensor("s2", [128, NB2 * 8, CALL2 // 16], i16,
                         kind="ExternalInput")
    m2d = nc.dram_tensor("m2", [128, NB2 * 8, 16], bf16,
                         kind="ExternalInput")
    xr = nc.dram_tensor("xr", [128, GG, 8], f32, kind="ExternalInput")
    ivd = nc.dram_tensor("ivd", [128, GG], f32, kind="ExternalInput")
    wd = nc.dram_tensor("wd", [64, 132], f32, kind="ExternalInput")
    wcd = nc.dram_tensor("wcd", [16, 64], f32, kind="ExternalInput")
    outd = nc.dram_tensor("outd", [S], f32, kind="ExternalOutput")

    with tile.TileContext(nc) as tc:
        with tc.tile_pool(name="sb", bufs=1) as sb, \
             tc.tile_pool(name="ps", bufs=1, space="PSUM") as ps, \
             tc.tile_pool(name="dram", bufs=1, space="DRAM") as dr:

            h1b = dr.tile([S // 2, 128], bf16, tag="h1b", name="h1b")
            h1f = dr.tile([C * S // 2, 128], bf16, tag="h1f", name="h1f")
            hTd = dr.tile([64, S], f32, tag="hTd", name="hTd")
            agg2 = dr.tile([S, 64], f32, tag="agg2", name="agg2")

            from concourse.masks import make_identity
            ident = sb.tile([128, 128], f32, tag="ident", name="ident")
            make_identity(nc, ident[:])
            wts = sb.tile([64, 132], f32, tag="wts", name="wts")
            nc.sync.dma_start(out=wts[:], in_=wd[:])
            wcomb = sb.tile([16, 64], f32, tag="wcomb", name="wcomb")
            nc.sync.dma_start(out=wcomb[:], in_=wcd[:])
            iot = sb.tile([128, 256], bf16, tag="iot", name="iot")
            nc.sync.dma_start(out=iot[:], in_=iotab[:])
            inv_sb = sb.tile([128, GG], f32, tag="inv", name="inv_sb")
            nc.sync.dma_start(out=inv_sb[:], in_=ivd[:])

            agg1 = sb.tile([128, GG, 4], f32, tag="agg1", name="agg1")
            nc.vector.memset(agg1[:], 0.0)

            # zero agg2 (32MB) early, contiguous 8KB runs per partition
            zb = sb.tile([128, 32, 64], f32, tag="zb", name="zb")
            nc.vector.memset(zb[:], 0.0)
            zrows = 128 * 32
            for base in range(0, S, zrows):
                n = min(zrows, S - base)
                nc.sync.dma_start(
                    out=agg2[base:base + n, :]
                    .rearrange("(p t) d -> p t d", p=128),
                    in_=zb[:, :n // 128, :])

            # ---------------- L1 aggregation ----------------
            iov = iot[:, :COLS1 * 32].rearrange("p (c v) -> p c v", c=COLS1)
            it1 = vt1 = None
            for k in range(ncalls1):
                if k % 8 == 0:
                    it1 = sb.tile([128, 8, CALL1 // 16], i16,
                                  tag=f"it1{(k // 8) % 2}", name=f"it1_{k}")
                    nc.sync.dma_start(out=it1[:], in_=idx1[:, k:k + 8, :])
                    vt1 = sb.tile([128, 8, COLS1], bf16,
                                  tag=f"vt1{(k // 8) % 2}", name=f"vt1_{k}")
                    nc.sync.dma_start(out=vt1[:], in_=vr1[:, k:k + 8, :])
                gb = sb.tile([128, COLS1, 128], bf16, tag=f"gb{k % 4}",
                             name=f"gb_{k}")
                nc.gpsimd.dma_gather(
                    out_ap=gb[:], in_ap=xtab[:, :],
                    idxs_ap=it1[:, k % 8, :],
                    num_idxs=CALL1, num_idxs_reg=CALL1, elem_size=128)
                oh = sb.tile([128, COLS1, 32], bf16, tag=f"oh{k % 2}",
                             name=f"oh_{k}")
                nc.vector.tensor_tensor(
                    out=oh[:], in0=iov,
                    in1=vt1[:, k % 8, :].unsqueeze(-1)
                    .broadcast_to([128, COLS1, 32]),
                    op=OP.is_equal)
                tt = sb.tile([128, COLS1, 32, 4], bf16, tag=f"tt{k % 2}",
                             name=f"tt_{k}")
                nc.vector.tensor_tensor(
                    out=tt[:],
                    in0=gb[:].rearrange("p c (v f) -> p c v f", f=4),
                    in1=oh[:].unsqueeze(-1).broadcast_to(
                        [128, COLS1, 32, 4]),
                    op=OP.mult)
                for (c0, c1, gg, accum) in frags.get(k, []):
                    src_ap = tt[:, c0:c1, :, :] \
                        .rearrange("p c v f -> p f (c v)")
                    if not accum:
                        nc.vector.tensor_reduce(
                            out=agg1[:, gg, :], in_=src_ap,
                            axis=mybir.AxisListType.X, op=OP.add)
                    else:
                        tmp = sb.tile([128, 4], f32, tag="rtmp",
                                      name=f"rtmp_{k}_{gg}")
                        nc.vector.tensor_reduce(
                            out=tmp[:], in_=src_ap,
                            axis=mybir.AxisListType.X, op=OP.add)
                        nc.vector.tensor_tensor(
                            out=agg1[:, gg, :], in0=agg1[:, gg, :],
                            in1=tmp[:], op=OP.add)

            # ---------------- L1 dense (+ chunked AllGather) -------------
            comb = []
            for i in range(2):
                t = sb.tile([128, 4, 16], f32, tag=f"comb{i}",
                            name=f"comb{i}")
                nc.vector.memset(t[:], 0.0)
                comb.append(t)
            agq = S // 8            # table elements per AllGather chunk
            for ch in range(nchunks):
                g0 = ch * 4
                cb = comb[ch % 2]
                nc.sync.dma_start(out=cb[:, :, 0:8],
                                  in_=xr[:, g0:g0 + 4, :])
                nc.vector.tensor_tensor(
                    out=cb[:, :, 8:12], in0=agg1[:, g0:g0 + 4, :],
                    in1=inv_sb[:, g0:g0 + 4].unsqueeze(-1)
                    .broadcast_to([128, 4, 4]),
                    op=OP.mult)
                pT = ps.tile([16, CHUNK], f32, tag="pT", name=f"pT{ch}")
                for t in range(4):
                    nc.tensor.transpose(
                        out=pT[:, t * 128:(t + 1) * 128],
                        in_=cb[:, t, :], identity=ident[:])
                cT = sb.tile([16, CHUNK], f32, tag="cT", name=f"cT{ch}")
                nc.scalar.copy(out=cT[:], in_=pT[:])
                pm = ps.tile([64, CHUNK], f32, tag="pm", name=f"pm{ch}")
                nc.tensor.matmul(pm[:], lhsT=wcomb[:], rhs=cT[:],
                                 start=True, stop=True)
                hT = sb.tile([64, CHUNK], f32, tag="hT", name=f"hT{ch}")
                nc.scalar.activation(hT[:], pm[:], AF.Relu,
                                     bias=wts[:, 129:130], scale=1.0)
                nc.sync.dma_start(out=hTd[:, ch * CHUNK:(ch + 1) * CHUNK],
                                  in_=hT[:])
                # pack bf16 pairs: element e=ch*256+2p+j holds nodes
                # (4p+2j, 4p+2j+1) of this chunk
                pb = ps.tile([128, 2, 128], f32, tag="pb", name=f"pb{ch}")
                hTv = hT[:].rearrange("f (c s) -> f s c", s=4)
                for j in range(2):
                    for h in range(2):
                        nc.tensor.transpose(
                            out=pb[:, j, 64 * h:64 * h + 64],
                            in_=hTv[:, 2 * j + h, :],
                            identity=ident[:64, :64])
                hb = sb.tile([128, 2, 128], bf16, tag="hb", name=f"hb{ch}")
                nc.vector.tensor_copy(out=hb[:], in_=pb[:])
                nc.sync.dma_start(
                    out=h1b[ch * 256:(ch + 1) * 256, :]
                    .rearrange("(p j) v -> p j v", p=128),
                    in_=hb[:])
                if (ch + 1) % (nchunks // 4) == 0:
                    r = (ch + 1) // (nchunks // 4) - 1
                    if os.environ.get("NO_CC"):
                        nc.sync.dma_start(
                            out=h1f[r * C * agq:r * C * agq + agq, :],
                            in_=h1b[r * agq:(r + 1) * agq, :])
                    else:
                        nc.gpsimd.collective_compute(
                            "AllGather", OP.bypass,
                            replica_groups=[list(range(C))],
                            ins=[h1b[r * agq:(r + 1) * agq, :]],
                            outs=[h1f[r * C * agq:(r + 1) * C * agq, :]])

            # ---------------- L2 aggregation ----------------
            it2 = st2 = mt2 = None
            for k in range(ncalls2):
                if k % 8 == 0:
                    it2 = sb.tile([128, 8, CALL2 // 16], i16,
                                  tag=f"it2{(k // 8) % 2}", name=f"it2_{k}")
                    nc.sync.dma_start(out=it2[:], in_=g2d[:, k:k + 8, :])
                    st2 = sb.tile([128, 8, CALL2 // 16], i16,
                                  tag=f"st2{(k // 8) % 2}", name=f"st2_{k}")
                    nc.sync.dma_start(out=st2[:], in_=s2d[:, k:k + 8, :])
                    mt2 = sb.tile([128, 8, 16], bf16,
                                  tag=f"mt2{(k // 8) % 2}", name=f"mt2_{k}")
                    nc.sync.dma_start(out=mt2[:], in_=m2d[:, k:k + 8, :])
                wbase, qbase = meta2[k]
                wrows = min(WIN2, T2 - wbase)
                gb2 = sb.tile([128, 8, 128], bf16, tag=f"gc{k % 4}",
                              name=f"gc_{k}")
                # slots CALL2..1023 are never written by the gather
                nc.vector.memset(gb2[CALL2 % 128:, 7, :], 0.0)
                nc.gpsimd.dma_gather(
                    out_ap=gb2[:], in_ap=h1f[wbase:wbase + wrows, :],
                    idxs_ap=it2[:, k % 8, :],
                    num_idxs=CALL2, num_idxs_reg=CALL2, elem_size=128)
                t2 = sb.tile([128, 8, 64, 2], bf16, tag=f"t2{k % 2}",
                             name=f"t2_{k}")
                nc.vector.tensor_tensor(
                    out=t2[:].rearrange("p c f two -> p c two f"),
                    in0=gb2[:].rearrange("p c (two f) -> p c two f", two=2),
                    in1=mt2[:, k % 8, :]
                    .rearrange("p (c two) -> p c two", two=2)
                    .unsqueeze(-1).broadcast_to([128, 8, 2, 64]),
                    op=OP.mult)
                ext = sb.tile([128, 8, 64], f32, tag=f"ex{k % 4}",
                              name=f"ex_{k}")
                nc.vector.tensor_reduce(
                    out=ext[:], in_=t2[:], axis=mybir.AxisListType.X,
                    op=OP.add)
                nc.gpsimd.dma_scatter_add(
                    out_ap=agg2[qbase:qbase + q, :],
                    in_ap=ext[:], idxs_ap=st2[:, k % 8, :],
                    num_idxs=CALL2, num_idxs_reg=CALL2, elem_size=64)

            # ---------------- L2 dense + head ----------------
            for ch in range(nchunks):
                g0 = ch * 4
                a2 = sb.tile([128, 4, 64], f32, tag=f"a2{ch % 2}",
                             name=f"a2_{ch}")
                nc.sync.dma_start(
                    out=a2[:],
                    in_=agg2[ch * CHUNK:(ch + 1) * CHUNK, :]
                    .rearrange("(p t) d -> p t d", p=128))
                nc.vector.tensor_tensor(
                    out=a2[:], in0=a2[:],
                    in1=inv_sb[:, g0:g0 + 4].unsqueeze(-1)
                    .broadcast_to([128, 4, 64]),
                    op=OP.mult)
                pa = ps.tile([64, CHUNK], f32, tag="pa", name=f"pa{ch}")
                for t in range(4):
                    nc.tensor.transpose(
                        out=pa[:, t * 128:(t + 1) * 128],
                        in_=a2[:, t, :], identity=ident[:])
                aT = sb.tile([64, CHUNK], f32, tag="aT", name=f"aT{ch}")
                nc.scalar.copy(out=aT[:], in_=pa[:])
                rT = sb.tile([64, CHUNK], f32, tag=f"rT{ch % 2}",
                             name=f"rT_{ch}")
                nc.sync.dma_start(
                    out=rT[:], in_=hTd[:, ch * CHUNK:(ch + 1) * CHUNK])
                pm2 = ps.tile([64, CHUNK], f32, tag="pm2", name=f"pm2_{ch}")
                nc.tensor.matmul(pm2[:], lhsT=wts[:, 0:64], rhs=aT[:],
                                 start=True, stop=False)
                nc.tensor.matmul(pm2[:], lhsT=wts[:, 64:128], rhs=rT[:],
                                 start=False, stop=True)
                h2 = sb.tile([64, CHUNK], f32, tag="h2", name=f"h2_{ch}")
                nc.scalar.activation(h2[:], pm2[:], AF.Relu,
                                     bias=wts[:, 130:131], scale=1.0)
                po = ps.tile([1, CHUNK], f32, tag="po", name=f"po{ch}")
                nc.tensor.matmul(po[:], lhsT=wts[:, 128:129], rhs=h2[:],
                                 start=True, stop=True)
                ob = sb.tile([1, CHUNK], f32, tag="ob", name=f"ob{ch}")
                nc.scalar.activation(ob[:], po[:], AF.Sigmoid,
                                     bias=wts[0:1, 131:132], scale=1.0)
                nc.sync.dma_start(
                    out=outd[ch * CHUNK:(ch + 1) * CHUNK]
                    .rearrange("(o c) -> o c", o=1),
                    in_=ob[:])

    nc.compile()
    return nc


# ---------------------------------------------------------------- entry

def _fold_weights(W1l, b1, W1r, g1, be1, rm1, rv1,
                  W2l, b2, W2r, g2, be2, rm2, rv2, Wp, bp):
    s1 = (np.asarray(g1) / np.sqrt(np.asarray(rv1) + EPS)).astype(np.float32)
    s2 = (np.asarray(g2) / np.sqrt(np.asarray(rv2) + EPS)).astype(np.float32)
    w1l = (s1[:, None] * np.asarray(W1l)).astype(np.float32)
    w1r = (s1[:, None] * np.asarray(W1r)).astype(np.float32)
    c1 = (np.asarray(be1) + (np.asarray(b1) - np.asarray(rm1)) * s1
          ).astype(np.float32)
    w2l = (s2[:, None] * np.asarray(W2l)).astype(np.float32)
    w2r = (s2[:, None] * np.asarray(W2r)).astype(np.float32)
    c2 = (np.asarray(be2) + (np.asarray(b2) - np.asarray(rm2)) * s2
          ).astype(np.float32)
    wts = np.zeros((64, 132), np.float32)
    wts[:, 0:64] = w2l.T
    wts[:, 64:128] = w2r.T
    wts[:, 128] = np.asarray(Wp, np.float32)[0]
    wts[:, 129] = c1
    wts[:, 130] = c2
    wts[0, 131] = np.float32(np.asarray(bp).ravel()[0])
    wcomb = np.zeros((16, 64), np.float32)
    wcomb[0:4, :] = w1r.T
    wcomb[8:12, :] = w1l.T
    return wts, wcomb


def _make_in_maps(x, sh, payload, pos_of_local, inv, wts, wcomb):
    S, GG = sh["S"], sh["GG"]
    NB1 = (sh["ncalls1"] + 7) // 8
    NB2 = (sh["ncalls2"] + 7) // 8
    xtab = np.zeros((XEL * PACK1, 4), BF)
    xtab[:N] = x.astype(BF)
    xtab = xtab.reshape(XEL, 128)
    iotab = np.tile(np.arange(32, dtype=BF), (128, 8)).reshape(128, 256)

    def padk(a, nb):
        # [ncalls, 128, X] -> [128, nb*8, X]
        full = np.zeros((nb * 8,) + a.shape[1:], a.dtype)
        full[: a.shape[0]] = a
        return np.ascontiguousarray(full.transpose(1, 0, 2))

    in_maps = []
    for c in range(C):
        pos = pos_of_local[c]
        xrp = np.zeros((S, 8), np.float32)
        xrp[pos, 0:4] = x[c * SLOC:(c + 1) * SLOC]
        ivp = np.zeros(S, np.float32)
        ivp[pos] = inv[c * SLOC:(c + 1) * SLOC]
        p = payload[c]
        in_maps.append({
            "xtab": xtab,
            "iotab": iotab,
            "idx1": padk(p["idx1"], NB1),
            "vr1": padk(p["vr1"], NB1),
            "g2": padk(p["g2"], NB2),
            "s2": padk(p["s2"], NB2),
            "m2": padk(p["m2"], NB2),
            "xr": np.ascontiguousarray(
                xrp.reshape(GG, 128, 8).transpose(1, 0, 2)),
            "ivd": np.ascontiguousarray(ivp.reshape(GG, 128).T),
            "wd": wts,
            "wcd": wcomb,
        })
    return in_maps


def _timed_run(nc, in_maps, iters):
    """Replicates bass2jax.run_bass_via_pjrt with pre-transferred inputs and
    no output donation, launching `iters` back-to-back executions to
    amortize dispatch latency. Returns (per-core results, per-call ns)."""
    import jax
    import concourse.mybir as mb
    from jax.sharding import Mesh, PartitionSpec, NamedSharding
    from jax.experimental.shard_map import shard_map
    from concourse import bass2jax
    bass2jax.install_neuronx_cc_hook()

    partition_name = (nc.partition_id_tensor.name
                      if nc.partition_id_tensor else None)
    in_names, out_names, out_avals = [], [], []
    for alloc in nc.m.functions[0].allocations:
        if not isinstance(alloc, mb.MemoryLocationSet):
            continue
        name = alloc.memorylocations[0].name
        if alloc.kind == "ExternalInput":
            if name != partition_name:
                in_names.append(name)
        elif alloc.kind == "ExternalOutput":
            out_names.append(name)
            out_avals.append(jax.core.ShapedArray(
                tuple(alloc.tensor_shape), mb.dt.np(alloc.dtype)))
    n_params = len(in_names)
    all_names = in_names + out_names + (
        [partition_name] if partition_name else [])

    def _body(*args):
        operands = list(args)
        if partition_name is not None:
            operands.append(bass2jax.partition_id_tensor())
        return tuple(bass2jax._bass_exec_p.bind(
            *operands, out_avals=tuple(out_avals),
            in_names=tuple(all_names), out_names=tuple(out_names),
            lowering_input_output_aliases=(),
            sim_require_finite=True, sim_require_nnan=True, nc=nc))

    devices = jax.devices()[:C]
    mesh = Mesh(np.asarray(devices), ("core",))
    nspec = n_params + len(out_names)
    donate = tuple(range(n_params, nspec))
    sharded = jax.jit(shard_map(
        _body, mesh=mesh, in_specs=(PartitionSpec("core"),) * nspec,
        out_specs=(PartitionSpec("core"),) * len(out_names),
        check_rep=False), donate_argnums=donate, keep_unused=True)
    shd = NamedSharding(mesh, PartitionSpec("core"))
    concat_in = [
        jax.device_put(np.concatenate(
            [np.asarray(in_maps[c][nm]) for c in range(C)], axis=0), shd)
        for nm in in_names]
    zero_sets = [
        [jax.device_put(
            np.zeros((C * av.shape[0], *av.shape[1:]), av.dtype), shd)
         for av in out_avals]
        for _ in range(iters + 1)]
    # warm-up (compiles / binds)
    outs = sharded(*concat_in, *zero_sets[0])
    jax.block_until_ready(outs)
    t0 = time.time()
    for i in range(iters):
        outs = sharded(*concat_in, *zero_sets[1 + i])
    jax.block_until_ready(outs)
    per_call = (time.time() - t0) / iters * 1e9
    res = [
        {nm: np.asarray(outs[i]).reshape(C, *out_avals[i].shape)[c]
         for i, nm in enumerate(out_names)}
        for c in range(C)]
    return res, per_call


def kernel(x, edge_index, W1l, b1, W1r, g1, be1, rm1, rv1,
           W2l, b2, W2r, g2, be2, rm2, rv2, Wp, bp, _sim=False):
    t0 = time.time()
    x = np.asarray(x, np.float32)
    edge_index = np.asarray(edge_index)
    src = edge_index[0].astype(np.int64)
    dst = edge_index[1].astype(np.int64)

    sh, payload, pos_of_local, inv = _host_prep(src, dst)
    S = sh["S"]
    print(f"[kernel] prep done: calls L1={sh['ncalls1']} "
          f"L2={sh['ncalls2']} S={S} {time.time() - t0:.0f}s", flush=True)

    wts, wcomb = _fold_weights(W1l, b1, W1r, g1, be1, rm1, rv1,
                               W2l, b2, W2r, g2, be2, rm2, rv2, Wp, bp)
    in_maps = _make_in_maps(x, sh, payload, pos_of_local, inv, wts, wcomb)
    print(f"[kernel] inputs packed {time.time() - t0:.0f}s", flush=True)

    nc = _build(sh)
    print(f"[kernel] build+compile done {time.time() - t0:.0f}s", flush=True)

    if _sim:
        import concourse.bass_interp as bass_interp
        sim = bass_interp.MultiCoreSim(nc, C)
        for d in range(C):
            for kk, v in in_maps[d].items():
                sim.cores[d].tensor(kk)[:] = np.asarray(v).reshape(
                    sim.cores[d].tensor(kk).shape)
        sim.simulate(check_with_hw=False)
        outs = [np.asarray(sim.cores[d].mem_tensor("outd")).reshape(S)
                [pos_of_local[d]] for d in range(C)]
        return np.concatenate(outs).reshape(N, 1).astype(np.float32)

    global LAST_EXEC_NS, LAST_RESULTS
    iters = int(os.environ.get("TIMER_K", "0"))
    if iters:
        results, per_call = _timed_run(nc, in_maps, iters)
        LAST_EXEC_NS = per_call
        LAST_RESULTS = results
        outs = [np.asarray(results[d]["outd"]).reshape(S)[pos_of_local[d]]
                for d in range(C)]
        return np.concatenate(outs).reshape(N, 1).astype(np.float32)
    t1 = time.time()
    res = run_bass_kernel_spmd(nc, in_maps, core_ids=list(range(C)),
                               trace=bool(int(os.environ.get("TRACE", "0"))))
    LAST_EXEC_NS = (time.time() - t1) * 1e9
    if res.exec_time_ns:
        LAST_EXEC_NS = res.exec_time_ns
    LAST_RESULTS = res
    outs = [np.asarray(res.results[d]["outd"]).reshape(S)[pos_of_local[d]]
            for d in range(C)]
    return np.concatenate(outs).reshape(N, 1).astype(np.float32)


# revision 21
# speedup vs baseline: 1.2675x; 1.2675x over previous
"""Trainium2 Bass kernel for the 2-layer GraphSAGE bus-stop predictor.

Self-contained: kernel(**inputs) -> np.ndarray [N, 1].

Strategy (8 NeuronCores, SPMD, dst-sharded nodes):

L1 aggregation (x is only 4-wide):
  - x packed bf16, 32 nodes per 256B gather element -> the whole 1M-node
    table is 31250 elements = ONE int16 window (8MB).
  - Per-core nodes sorted into rectangular [128 x d] degree-class groups
    (d in {1,2,4,8,16,32}); gather calls stream group columns; an on-chip
    one-hot (iota==v) select + fused reduce sums each group's columns
    straight into an SBUF-resident agg1 accumulator. NO scatter-add.
L2 aggregation (h1 is 64-wide):
  - h1 stored bf16 paired (2 nodes per 256B element); chunked AllGather
    builds the global table (16 windows x 4 dst-quarters packing cells).
  - gather pair -> on-chip mask blend extracts the right half ->
    dma_scatter_add (CCE) into fp32 agg2 in HBM.
Dense phases: BN folded into weights; L1 uses a combined [x|agg] K=16
matmul; L2 keeps h1 feature-major in DRAM so the root term needs no
transpose; the 1-wide head + sigmoid is fused into the L2 loop.
"""

import os
import time

import numpy as np
import ml_dtypes

import concourse.bacc as bacc
import concourse.mybir as mybir
import concourse.tile as tile
from concourse.bass_utils import run_bass_kernel_spmd

f32 = mybir.dt.float32
bf16 = mybir.dt.bfloat16
i16 = mybir.dt.int16
BF = ml_dtypes.bfloat16

AF = mybir.ActivationFunctionType
OP = mybir.AluOpType

N = 1_000_000
C = 8                    # cores
SLOC = N // C            # nodes per core
PACK1 = 32               # x nodes per 256B gather element
XEL = N // PACK1         # 31250 L1 table elements (single int16 window)
# SWDGE call sizes. The IndirectLoad completion-semaphore wait is
# num_idxs * elem_bytes/4 (+4) and must fit 16 bits -> 1024x256B is
# exactly over. L1 keeps full 128-row columns (7 per call); L2 is flat.
CALL1 = 896
COLS1 = 7
CALL2 = 960
WIN2 = 32768             # L2 gather window (elements)
CHUNK = 512              # dense-phase nodes per chunk
CLS = (1, 2, 4, 8, 16, 32)   # padded degree classes (0 handled separately)
PADV = 200.0             # one-hot sentinel: matches no iota value
EPS = 1e-5

LAST_EXEC_NS = None
LAST_RESULTS = None


# ---------------------------------------------------------------- host prep

def _wrap_idx(vals, call):
    """int16 vals [k, call] -> dma idx tiles [k, 128, call//16]."""
    m = np.asarray(vals, dtype=np.int16).reshape(-1, call)
    k = m.shape[0]
    out = np.zeros((k, 128, call // 16), dtype=np.int16)
    ii = np.arange(call)
    for g in range(8):
        out[:, (ii % 16) + 16 * g, ii // 16] = m
    return out


def _agg2_row(pos):
    """Position -> agg2 row (in-chunk interleave so dense loads have
    1KB-contiguous runs per partition)."""
    pm = pos % CHUNK
    return (pos // CHUNK) * CHUNK + (pm % 128) * 4 + pm // 128


def _host_prep(src, dst):
    deg = np.bincount(dst, minlength=N).astype(np.int64)
    inv = (1.0 / np.maximum(deg, 1.0)).astype(np.float32)
    assert int(deg.max()) <= CLS[-1], int(deg.max())

    degc = deg.reshape(C, SLOC)
    pcls = np.zeros((C, SLOC), np.int64)
    lo = 0
    for k in CLS:
        pcls[(degc > lo) & (degc <= k)] = k
        lo = k

    cnt = {k: np.array([(pcls[c] == k).sum() for c in range(C)]) for k in CLS}
    G = {k: int(np.ceil((cnt[k].max() + 4) / 128)) for k in CLS}
    rows_used = 128 * sum(G.values())
    cnt0 = np.array([(pcls[c] == 0).sum() for c in range(C)])
    rows0 = int(np.ceil((cnt0.max() + 4) / 128)) * 128
    S = rows_used + rows0
    S = ((S + 2047) // 2048) * 2048
    q = S // 4
    assert q <= 32767, S
    reserved = np.array([q - 1, 2 * q - 1, 3 * q - 1, 4 * q - 1])
    GG = S // 128
    classbase = {}
    b = 0
    for k in CLS:
        classbase[k] = b
        b += 128 * G[k]
    zerobase = b

    # per-core incoming-edge CSR (by dst) and node positions
    pos_of_local = []
    core_csr = []
    for c in range(C):
        m = (dst // SLOC) == c
        sd = src[m]
        dl = dst[m] - c * SLOC
        o = np.argsort(dl, kind="stable")
        sd = sd[o]
        starts = np.searchsorted(dl[o], np.arange(SLOC + 1))
        core_csr.append((sd, starts))
        pos = np.full(SLOC, -1, np.int64)
        for k in CLS + (0,):
            nodes = np.nonzero(pcls[c] == k)[0]
            base = classbase[k] if k else zerobase
            length = 128 * G[k] if k else S - zerobase
            pr = base + np.arange(length)
            pr = pr[~np.isin(pr, reserved)][: len(nodes)]
            assert len(pr) == len(nodes), (c, k, len(nodes))
            pos[nodes] = pr
        pos_of_local.append(pos)

    # ---- L1 gather streams + shared reduce-fragment schedule
    frags = {}           # call -> list[(c0, c1, gg, accum)]
    callbase = {}
    ncalls1 = 0
    for k in CLS:
        callbase[k] = ncalls1
        ncalls1 += (G[k] * k + COLS1 - 1) // COLS1
        for g in range(G[k]):
            gg = classbase[k] // 128 + g
            c0 = g * k
            first = True
            while c0 < (g + 1) * k:
                call = callbase[k] + c0 // COLS1
                e = min((g + 1) * k, (c0 // COLS1 + 1) * COLS1)
                frags.setdefault(call, []).append(
                    (c0 % COLS1, ((e - 1) % COLS1) + 1, gg, not first))
                first = False
                c0 = e

    idx1 = [[] for _ in range(C)]
    vr1 = [[] for _ in range(C)]
    for c in range(C):
        sd, starts = core_csr[c]
        lens = np.diff(starts)
        pos = pos_of_local[c]
        for k in CLS:
            nodes = np.nonzero(pcls[c] == k)[0]
            rows = np.full(G[k] * 128, -1, np.int64)
            rows[pos[nodes] - classbase[k]] = nodes
            srcmat = np.full((G[k] * 128, k), -1, np.int64)
            nz = np.nonzero(rows >= 0)[0]
            nd = rows[nz]
            if len(nd):
                ln = lens[nd]
                rr = np.repeat(nz, ln)
                cc2 = (np.arange(len(rr)) -
                       np.repeat(np.cumsum(
                           np.concatenate([[0], ln[:-1]])), ln))
                flat = np.repeat(starts[nd], ln) + cc2
                srcmat[rr, cc2] = sd[flat]
            st = srcmat.reshape(G[k], 128, k).transpose(0, 2, 1).reshape(-1)
            ncall_k = (G[k] * k + COLS1 - 1) // COLS1
            padn = ncall_k * CALL1 - len(st)
            st = np.concatenate([st, np.full(padn, -1, np.int64)])
            gi = np.where(st >= 0, st // PACK1, 0).astype(np.int16)
            vv = np.where(st >= 0, st % PACK1, PADV).astype(BF)
            idx1[c].append(gi.reshape(-1, CALL1))
            vr1[c].append(vv.reshape(-1, COLS1, 128).transpose(0, 2, 1))
        idx1[c] = _wrap_idx(np.concatenate(idx1[c]), CALL1)
        vr1[c] = np.concatenate(vr1[c]).astype(BF)
    assert idx1[0].shape[0] == ncalls1

    # ---- L2 cell streams
    T2 = C * S // 2
    nwin = (T2 + WIN2 - 1) // WIN2
    cells = [dict() for _ in range(C)]
    dump_row = _agg2_row(q - 1) % q
    for c in range(C):
        sd, starts = core_csr[c]
        lens = np.diff(starts)
        pos = pos_of_local[c]
        dposn = np.repeat(pos, lens)
        sc = sd // SLOC
        agq = S // 8           # elements per core per AllGather chunk
        elem = np.empty(len(sd), np.int64)
        half = np.empty(len(sd), np.int64)
        ag_shared = bool(os.environ.get("AG_SHARED"))
        for cc in range(C):
            m2 = sc == cc
            pp = pos_of_local[cc][sd[m2] % SLOC]
            el = pp // 2
            if ag_shared:
                # single AllGather output is core-major
                elem[m2] = cc * (S // 2) + el
            else:
                # h1f is chunk-major: [chunk r][core cc][rows] so each
                # chunked AllGather output is contiguous
                elem[m2] = (el // agq) * (C * agq) + cc * agq + el % agq
            half[m2] = pp % 2
        w = elem // WIN2
        drow = _agg2_row(dposn)
        qq = drow // q
        sidx = drow % q
        order = np.lexsort((sidx, qq, w))
        elem, half, w, qq, sidx = (a[order] for a in (elem, half, w, qq, sidx))
        bounds = np.searchsorted(w * 4 + qq, np.arange(nwin * 4 + 1))
        for cell in range(nwin * 4):
            lo2, hi = bounds[cell], bounds[cell + 1]
            if lo2 == hi:
                continue
            cw, cq = cell // 4, cell % 4
            ge = (elem[lo2:hi] - cw * WIN2).astype(np.int16)
            hh = half[lo2:hi].astype(np.int16)
            ds = sidx[lo2:hi].astype(np.int16)
            calls = []
            pend = list(zip(ge.tolist(), hh.tolist(), ds.tolist()))
            while pend:
                cg, ch, cd = [], [], []
                seen = set()
                nxt = []
                for gg2, hh2, dd2 in pend:
                    if len(cg) < CALL2 and dd2 not in seen:
                        cg.append(gg2)
                        ch.append(hh2)
                        cd.append(dd2)
                        seen.add(dd2)
                    else:
                        nxt.append((gg2, hh2, dd2))
                while len(cg) < CALL2:
                    cg.append(0)
                    ch.append(-1)
                    cd.append(dump_row)
                calls.append((np.array(cg, np.int16), np.array(ch, np.int16),
                              np.array(cd, np.int16)))
                pend = nxt
            cells[c][(cw, cq)] = calls

    recs = []
    for cell in sorted({kk for cc in cells for kk in cc}):
        ncall = max(len(cc.get(cell, [])) for cc in cells)
        cw, cq = cell
        for kk in range(ncall):
            recs.append((kk, cw, cq))
    recs.sort()
    meta2 = []
    g2 = [[] for _ in range(C)]
    s2 = [[] for _ in range(C)]
    m2l = [[] for _ in range(C)]
    for kk, cw, cq in recs:
        meta2.append((cw * WIN2, cq))
        for c in range(C):
            cl = cells[c].get((cw, cq), [])
            if kk < len(cl):
                cg, ch, cd = cl[kk]
            else:
                cg = np.zeros(CALL2, np.int16)
                ch = np.full(CALL2, -1, np.int16)
                cd = np.full(CALL2, dump_row, np.int16)
            g2[c].append(cg)
            s2[c].append(cd)
            mm = np.zeros((1024, 2), BF)
            mm[:CALL2][ch == 0, 0] = 1
            mm[:CALL2][ch == 1, 1] = 1
            m2l[c].append(mm.reshape(8, 128, 2).transpose(1, 0, 2))
    ncalls2 = len(meta2)
    for c in range(C):
        g2[c] = _wrap_idx(np.concatenate(g2[c]), CALL2)
        s2[c] = _wrap_idx(np.concatenate(s2[c]), CALL2)
        m2l[c] = np.stack(m2l[c]).reshape(ncalls2, 128, 16)

    shared = dict(S=S, q=q, GG=GG, ncalls1=ncalls1, frags=frags,
                  meta2=meta2, ncalls2=ncalls2, T2=T2)
    payload = [dict(idx1=idx1[c], vr1=vr1[c], g2=g2[c], s2=s2[c], m2=m2l[c])
               for c in range(C)]
    return shared, payload, pos_of_local, inv


# ---------------------------------------------------------------- bass build

def _build(sh):
    S, q, GG = sh["S"], sh["q"], sh["GG"]
    ncalls1, ncalls2 = sh["ncalls1"], sh["ncalls2"]
    frags, meta2, T2 = sh["frags"], sh["meta2"], sh["T2"]
    nchunks = S // CHUNK
    NB1 = (ncalls1 + 7) // 8
    NB2 = (ncalls2 + 7) // 8

    nc = bacc.Bacc("TRN2", target_bir_lowering=False, debug=False,
                   num_devices=C, num_swdge_queues=4)
    xtab = nc.dram_tensor("xtab", [XEL, 128], bf16, kind="ExternalInput")
    iotab = nc.dram_tensor("iotab", [128, 256], bf16, kind="ExternalInput")
    idx1 = nc.dram_tensor("idx1", [128, NB1 * 8, CALL1 // 16], i16,
                          kind="ExternalInput")
    vr1 = nc.dram_tensor("vr1", [128, NB1 * 8, COLS1], bf16,
                         kind="ExternalInput")
    g2d = nc.dram_tensor("g2", [128, NB2 * 8, CALL2 // 16], i16,
                         kind="ExternalInput")
    s2d = nc.dram_tensor("s2", [128, NB2 * 8, CALL2 // 16], i16,
                         kind="ExternalInput")
    m2d = nc.dram_tensor("m2", [128, NB2 * 8, 16], bf16,
                         kind="ExternalInput")
    xr = nc.dram_tensor("xr", [128, GG, 8], f32, kind="ExternalInput")
    ivd = nc.dram_tensor("ivd", [128, GG], f32, kind="ExternalInput")
    wd = nc.dram_tensor("wd", [64, 132], f32, kind="ExternalInput")
    wcd = nc.dram_tensor("wcd", [16, 64], f32, kind="ExternalInput")
    outd = nc.dram_tensor("outd", [S], f32, kind="ExternalOutput")

    with tile.TileContext(nc) as tc:
        with tc.tile_pool(name="sb", bufs=1) as sb, \
             tc.tile_pool(name="ps", bufs=1, space="PSUM") as ps, \
             tc.tile_pool(name="dram", bufs=1, space="DRAM") as dr:

            h1b = dr.tile([S // 2, 128], bf16, tag="h1b", name="h1b")
            ag_shared = bool(os.environ.get("AG_SHARED"))
            h1f = dr.tile([C * S // 2, 128], bf16, tag="h1f", name="h1f",
                          addr_space="Shared" if ag_shared else "Local")
            hTd = dr.tile([64, S], f32, tag="hTd", name="hTd")
            agg2q = [dr.tile([q, 64], f32, tag=f"agg2q{i}",
                             name=f"agg2q{i}") for i in range(4)]

            from concourse.masks import make_identity
            ident = sb.tile([128, 128], f32, tag="ident", name="ident")
            make_identity(nc, ident[:])
            wts = sb.tile([64, 132], f32, tag="wts", name="wts")
            nc.sync.dma_start(out=wts[:], in_=wd[:])
            wcomb = sb.tile([16, 64], f32, tag="wcomb", name="wcomb")
            nc.sync.dma_start(out=wcomb[:], in_=wcd[:])
            iot = sb.tile([128, 256], bf16, tag="iot", name="iot")
            nc.sync.dma_start(out=iot[:], in_=iotab[:])
            inv_sb = sb.tile([128, GG], f32, tag="inv", name="inv_sb")
            nc.sync.dma_start(out=inv_sb[:], in_=ivd[:])

            agg1 = sb.tile([128, GG, 4], f32, tag="agg1", name="agg1")
            nc.vector.memset(agg1[:], 0.0)

            # zero agg2 (32MB) early, contiguous 8KB runs per partition
            zb = sb.tile([128, 32, 64], f32, tag="zb", name="zb")
            nc.vector.memset(zb[:], 0.0)
            zrows = 128 * 32
            for qi in range(4):
                for base in range(0, q, zrows):
                    n = min(zrows, q - base)
                    nc.sync.dma_start(
                        out=agg2q[qi][base:base + n, :]
                        .rearrange("(p t) d -> p t d", p=128),
                        in_=zb[:, :n // 128, :])

            # ---------------- L1 aggregation ----------------
            # SWDGE sem lanes are assigned round-robin in EMISSION order
            # (mod 8); queue_num must stay consistent with the lane, so
            # queue = emission index % 4 (8 % 4 == 0 keeps lane->queue 1:1)
            swop = [0]

            def nq():
                v = swop[0] % 4
                swop[0] += 1
                return v

            iov = iot[:, :COLS1 * 32].rearrange("p (c v) -> p c v", c=COLS1)
            it1 = vt1 = None
            for k in range(0 if os.environ.get("NO_L1") else ncalls1):
                if k % 8 == 0:
                    it1 = sb.tile([128, 8, CALL1 // 16], i16,
                                  tag=f"it1{(k // 8) % 2}", name=f"it1_{k}")
                    nc.sync.dma_start(out=it1[:], in_=idx1[:, k:k + 8, :])
                    vt1 = sb.tile([128, 8, COLS1], bf16,
                                  tag=f"vt1{(k // 8) % 2}", name=f"vt1_{k}")
                    nc.sync.dma_start(out=vt1[:], in_=vr1[:, k:k + 8, :])
                gb = sb.tile([128, COLS1, 128], bf16, tag=f"gb{k % 4}",
                             name=f"gb_{k}")
                nc.gpsimd.dma_gather(
                    out_ap=gb[:], in_ap=xtab[:, :],
                    idxs_ap=it1[:, k % 8, :],
                    num_idxs=CALL1, num_idxs_reg=CALL1, elem_size=128,
                    queue_num=nq())
                oh = sb.tile([128, COLS1, 32], bf16, tag=f"oh{k % 2}",
                             name=f"oh_{k}")
                nc.vector.tensor_tensor(
                    out=oh[:], in0=iov,
                    in1=vt1[:, k % 8, :].unsqueeze(-1)
                    .broadcast_to([128, COLS1, 32]),
                    op=OP.is_equal)
                tt = sb.tile([128, COLS1, 32, 4], bf16, tag=f"tt{k % 2}",
                             name=f"tt_{k}")
                nc.vector.tensor_tensor(
                    out=tt[:],
                    in0=gb[:].rearrange("p c (v f) -> p c v f", f=4),
                    in1=oh[:].unsqueeze(-1).broadcast_to(
                        [128, COLS1, 32, 4]),
                    op=OP.mult)
                for (c0, c1, gg, accum) in frags.get(k, []):
                    src_ap = tt[:, c0:c1, :, :] \
                        .rearrange("p c v f -> p f (c v)")
                    if not accum:
                        nc.vector.tensor_reduce(
                            out=agg1[:, gg, :], in_=src_ap,
                            axis=mybir.AxisListType.X, op=OP.add)
                    else:
                        tmp = sb.tile([128, 4], f32, tag="rtmp",
                                      name=f"rtmp_{k}_{gg}")
                        nc.vector.tensor_reduce(
                            out=tmp[:], in_=src_ap,
                            axis=mybir.AxisListType.X, op=OP.add)
                        nc.vector.tensor_tensor(
                            out=agg1[:, gg, :], in0=agg1[:, gg, :],
                            in1=tmp[:], op=OP.add)

            # ---------------- L1 dense (+ chunked AllGather) -------------
            comb = []
            for i in range(2):
                t = sb.tile([128, 4, 16], f32, tag=f"comb{i}",
                            name=f"comb{i}")
                nc.vector.memset(t[:], 0.0)
                comb.append(t)
            agq = S // 8            # table elements per AllGather chunk
            for ch in range(nchunks):
                g0 = ch * 4
                cb = comb[ch % 2]
                nc.sync.dma_start(out=cb[:, :, 0:8],
                                  in_=xr[:, g0:g0 + 4, :])
                nc.vector.tensor_tensor(
                    out=cb[:, :, 8:12], in0=agg1[:, g0:g0 + 4, :],
                    in1=inv_sb[:, g0:g0 + 4].unsqueeze(-1)
                    .broadcast_to([128, 4, 4]),
                    op=OP.mult)
                pT = ps.tile([16, CHUNK], f32, tag="pT", name=f"pT{ch}")
                for t in range(4):
                    nc.tensor.transpose(
                        out=pT[:, t * 128:(t + 1) * 128],
                        in_=cb[:, t, :], identity=ident[:])
                cT = sb.tile([16, CHUNK], f32, tag="cT", name=f"cT{ch}")
                nc.scalar.copy(out=cT[:], in_=pT[:])
                pm = ps.tile([64, CHUNK], f32, tag="pm", name=f"pm{ch}")
                nc.tensor.matmul(pm[:], lhsT=wcomb[:], rhs=cT[:],
                                 start=True, stop=True)
                hT = sb.tile([64, CHUNK], f32, tag="hT", name=f"hT{ch}")
                nc.scalar.activation(hT[:], pm[:], AF.Relu,
                                     bias=wts[:, 129:130], scale=1.0)
                nc.sync.dma_start(out=hTd[:, ch * CHUNK:(ch + 1) * CHUNK],
                                  in_=hT[:])
                # pack bf16 pairs: element e=ch*256+2p+j holds nodes
                # (4p+2j, 4p+2j+1) of this chunk
                pb = ps.tile([128, 2, 128], f32, tag="pb", name=f"pb{ch}")
                hTv = hT[:].rearrange("f (c s) -> f s c", s=4)
                for j in range(2):
                    for h in range(2):
                        nc.tensor.transpose(
                            out=pb[:, j, 64 * h:64 * h + 64],
                            in_=hTv[:, 2 * j + h, :],
                            identity=ident[:64, :64])
                hb = sb.tile([128, 2, 128], bf16, tag="hb", name=f"hb{ch}")
                nc.vector.tensor_copy(out=hb[:], in_=pb[:])
                nc.sync.dma_start(
                    out=h1b[ch * 256:(ch + 1) * 256, :]
                    .rearrange("(p j) v -> p j v", p=128),
                    in_=hb[:])
                if ag_shared:
                    # Shared output allows only one writer: single AllGather
                    # of the whole table, core-major (host mapping matches)
                    if ch + 1 == nchunks:
                        nc.gpsimd.collective_compute(
                            "AllGather", OP.bypass,
                            replica_groups=[list(range(C))],
                            ins=[h1b[:]],
                            outs=[h1f[:]])
                elif (ch + 1) % (nchunks // 4) == 0:
                    r = (ch + 1) // (nchunks // 4) - 1
                    if os.environ.get("NO_CC"):
                        nc.sync.dma_start(
                            out=h1f[r * C * agq:r * C * agq + agq, :],
                            in_=h1b[r * agq:(r + 1) * agq, :])
                    else:
                        nc.gpsimd.collective_compute(
                            "AllGather", OP.bypass,
                            replica_groups=[list(range(C))],
                            ins=[h1b[r * agq:(r + 1) * agq, :]],
                            outs=[h1f[r * C * agq:(r + 1) * C * agq, :]])

            # ---------------- L2 aggregation ----------------
            it2 = st2 = mt2 = None
            exts = {}
            nc2 = 0 if os.environ.get("NO_L2") else ncalls2
            for kb in range(0, nc2, 4):
                blk = range(kb, min(kb + 4, nc2))
                for k in blk:
                    if k % 8 == 0:
                        it2 = sb.tile([128, 8, CALL2 // 16], i16,
                                      tag=f"it2{(k // 8) % 2}",
                                      name=f"it2_{k}")
                        nc.sync.dma_start(out=it2[:], in_=g2d[:, k:k + 8, :])
                        st2 = sb.tile([128, 8, CALL2 // 16], i16,
                                      tag=f"st2{(k // 8) % 2}",
                                      name=f"st2_{k}")
                        nc.sync.dma_start(out=st2[:], in_=s2d[:, k:k + 8, :])
                        mt2 = sb.tile([128, 8, 16], bf16,
                                      tag=f"mt2{(k // 8) % 2}",
                                      name=f"mt2_{k}")
                        nc.sync.dma_start(out=mt2[:], in_=m2d[:, k:k + 8, :])
                    wbase, _qq = meta2[k]
                    wrows = min(WIN2, T2 - wbase)
                    gb2 = sb.tile([128, 8, 128], bf16, tag=f"gc{k % 4}",
                                  name=f"gc_{k}")
                    # slots CALL2..1023 are never written by the gather
                    nc.vector.memset(gb2[CALL2 % 128:, 7, :], 0.0)
                    nc.gpsimd.dma_gather(
                        out_ap=gb2[:], in_ap=h1f[wbase:wbase + wrows, :],
                        idxs_ap=it2[:, k % 8, :],
                        num_idxs=CALL2, num_idxs_reg=CALL2, elem_size=128,
                        queue_num=nq())
                    t2 = sb.tile([128, 8, 64, 2], bf16, tag=f"t2{k % 2}",
                                 name=f"t2_{k}")
                    nc.vector.tensor_tensor(
                        out=t2[:].rearrange("p c f two -> p c two f"),
                        in0=gb2[:].rearrange("p c (two f) -> p c two f",
                                             two=2),
                        in1=mt2[:, k % 8, :]
                        .rearrange("p (c two) -> p c two", two=2)
                        .unsqueeze(-1).broadcast_to([128, 8, 2, 64]),
                        op=OP.mult)
                    ext = sb.tile([128, 8, 64], f32, tag=f"ex{k % 4}",
                                  name=f"ex_{k}")
                    nc.vector.tensor_reduce(
                        out=ext[:], in_=t2[:], axis=mybir.AxisListType.X,
                        op=OP.add)
                    exts[k] = (ext, st2)
                if not os.environ.get("NO_SCAT"):
                    for k in blk:
                        ext, stt = exts.pop(k)
                        nc.gpsimd.dma_scatter_add(
                            out_ap=agg2q[meta2[k][1]][0:q, :],
                            in_ap=ext[:], idxs_ap=stt[:, k % 8, :],
                            num_idxs=CALL2, num_idxs_reg=CALL2, elem_size=64,
                            queue_num=nq())

            # ---------------- L2 dense + head ----------------
            for ch in range(nchunks):
                g0 = ch * 4
                a2 = sb.tile([128, 4, 64], f32, tag=f"a2{ch % 2}",
                             name=f"a2_{ch}")
                qi = (ch * CHUNK) // q
                lo = ch * CHUNK - qi * q
                nc.sync.dma_start(
                    out=a2[:],
                    in_=agg2q[qi][lo:lo + CHUNK, :]
                    .rearrange("(p t) d -> p t d", p=128))
                nc.vector.tensor_tensor(
                    out=a2[:], in0=a2[:],
                    in1=inv_sb[:, g0:g0 + 4].unsqueeze(-1)
                    .broadcast_to([128, 4, 64]),
                    op=OP.mult)
                pa = ps.tile([64, CHUNK], f32, tag="pa", name=f"pa{ch}")
                for t in range(4):
                    nc.tensor.transpose(
                        out=pa[:, t * 128:(t + 1) * 128],
                        in_=a2[:, t, :], identity=ident[:])
                aT = sb.tile([64, CHUNK], f32, tag="aT", name=f"aT{ch}")
                nc.scalar.copy(out=aT[:], in_=pa[:])
                rT = sb.tile([64, CHUNK], f32, tag=f"rT{ch % 2}",
                             name=f"rT_{ch}")
                nc.sync.dma_start(
                    out=rT[:], in_=hTd[:, ch * CHUNK:(ch + 1) * CHUNK])
                pm2 = ps.tile([64, CHUNK], f32, tag="pm2", name=f"pm2_{ch}")
                nc.tensor.matmul(pm2[:], lhsT=wts[:, 0:64], rhs=aT[:],
                                 start=True, stop=False)
                nc.tensor.matmul(pm2[:], lhsT=wts[:, 64:128], rhs=rT[:],
                                 start=False, stop=True)
                h2 = sb.tile([64, CHUNK], f32, tag="h2", name=f"h2_{ch}")
                nc.scalar.activation(h2[:], pm2[:], AF.Relu,
                                     bias=wts[:, 130:131], scale=1.0)
                po = ps.tile([1, CHUNK], f32, tag="po", name=f"po{ch}")
                nc.tensor.matmul(po[:], lhsT=wts[:, 128:129], rhs=h2[:],
                                 start=True, stop=True)
                ob = sb.tile([1, CHUNK], f32, tag="ob", name=f"ob{ch}")
                nc.scalar.activation(ob[:], po[:], AF.Sigmoid,
                                     bias=wts[0:1, 131:132], scale=1.0)
                nc.sync.dma_start(
                    out=outd[ch * CHUNK:(ch + 1) * CHUNK]
                    .rearrange("(o c) -> o c", o=1),
                    in_=ob[:])

    nc.compile()
    return nc


# ---------------------------------------------------------------- entry

def _fold_weights(W1l, b1, W1r, g1, be1, rm1, rv1,
                  W2l, b2, W2r, g2, be2, rm2, rv2, Wp, bp):
    s1 = (np.asarray(g1) / np.sqrt(np.asarray(rv1) + EPS)).astype(np.float32)
    s2 = (np.asarray(g2) / np.sqrt(np.asarray(rv2) + EPS)).astype(np.float32)
    w1l = (s1[:, None] * np.asarray(W1l)).astype(np.float32)
    w1r = (s1[:, None] * np.asarray(W1r)).astype(np.float32)
    c1 = (np.asarray(be1) + (np.asarray(b1) - np.asarray(rm1)) * s1
          ).astype(np.float32)
    w2l = (s2[:, None] * np.asarray(W2l)).astype(np.float32)
    w2r = (s2[:, None] * np.asarray(W2r)).astype(np.float32)
    c2 = (np.asarray(be2) + (np.asarray(b2) - np.asarray(rm2)) * s2
          ).astype(np.float32)
    wts = np.zeros((64, 132), np.float32)
    wts[:, 0:64] = w2l.T
    wts[:, 64:128] = w2r.T
    wts[:, 128] = np.asarray(Wp, np.float32)[0]
    wts[:, 129] = c1
    wts[:, 130] = c2
    wts[0, 131] = np.float32(np.asarray(bp).ravel()[0])
    wcomb = np.zeros((16, 64), np.float32)
    wcomb[0:4, :] = w1r.T
    wcomb[8:12, :] = w1l.T
    return wts, wcomb


def _make_in_maps(x, sh, payload, pos_of_local, inv, wts, wcomb):
    S, GG = sh["S"], sh["GG"]
    NB1 = (sh["ncalls1"] + 7) // 8
    NB2 = (sh["ncalls2"] + 7) // 8
    xtab = np.zeros((XEL * PACK1, 4), BF)
    xtab[:N] = x.astype(BF)
    xtab = xtab.reshape(XEL, 128)
    iotab = np.tile(np.arange(32, dtype=BF), (128, 8)).reshape(128, 256)

    def padk(a, nb):
        # [ncalls, 128, X] -> [128, nb*8, X]
        full = np.zeros((nb * 8,) + a.shape[1:], a.dtype)
        full[: a.shape[0]] = a
        return np.ascontiguousarray(full.transpose(1, 0, 2))

    in_maps = []
    for c in range(C):
        pos = pos_of_local[c]
        xrp = np.zeros((S, 8), np.float32)
        xrp[pos, 0:4] = x[c * SLOC:(c + 1) * SLOC]
        ivp = np.zeros(S, np.float32)
        ivp[pos] = inv[c * SLOC:(c + 1) * SLOC]
        p = payload[c]
        in_maps.append({
            "xtab": xtab,
            "iotab": iotab,
            "idx1": padk(p["idx1"], NB1),
            "vr1": padk(p["vr1"], NB1),
            "g2": padk(p["g2"], NB2),
            "s2": padk(p["s2"], NB2),
            "m2": padk(p["m2"], NB2),
            "xr": np.ascontiguousarray(
                xrp.reshape(GG, 128, 8).transpose(1, 0, 2)),
            "ivd": np.ascontiguousarray(ivp.reshape(GG, 128).T),
            "wd": wts,
            "wcd": wcomb,
        })
    return in_maps


def _timed_run(nc, in_maps, iters):
    """Replicates bass2jax.run_bass_via_pjrt with pre-transferred inputs and
    no output donation, launching `iters` back-to-back executions to
    amortize dispatch latency. Returns (per-core results, per-call ns)."""
    import jax
    import concourse.mybir as mb
    from jax.sharding import Mesh, PartitionSpec, NamedSharding
    from jax.experimental.shard_map import shard_map
    from concourse import bass2jax
    bass2jax.install_neuronx_cc_hook()

    partition_name = (nc.partition_id_tensor.name
                      if nc.partition_id_tensor else None)
    in_names, out_names, out_avals = [], [], []
    for alloc in nc.m.functions[0].allocations:
        if not isinstance(alloc, mb.MemoryLocationSet):
            continue
        name = alloc.memorylocations[0].name
        if alloc.kind == "ExternalInput":
            if name != partition_name:
                in_names.append(name)
        elif alloc.kind == "ExternalOutput":
            out_names.append(name)
            out_avals.append(jax.core.ShapedArray(
                tuple(alloc.tensor_shape), mb.dt.np(alloc.dtype)))
    n_params = len(in_names)
    all_names = in_names + out_names + (
        [partition_name] if partition_name else [])

    def _body(*args):
        operands = list(args)
        if partition_name is not None:
            operands.append(bass2jax.partition_id_tensor())
        return tuple(bass2jax._bass_exec_p.bind(
            *operands, out_avals=tuple(out_avals),
            in_names=tuple(all_names), out_names=tuple(out_names),
            lowering_input_output_aliases=(),
            sim_require_finite=True, sim_require_nnan=True, nc=nc))

    devices = jax.devices()[:C]
    mesh = Mesh(np.asarray(devices), ("core",))
    nspec = n_params + len(out_names)
    donate = tuple(range(n_params, nspec))
    sharded = jax.jit(shard_map(
        _body, mesh=mesh, in_specs=(PartitionSpec("core"),) * nspec,
        out_specs=(PartitionSpec("core"),) * len(out_names),
        check_rep=False), donate_argnums=donate, keep_unused=True)
    shd = NamedSharding(mesh, PartitionSpec("core"))
    concat_in = [
        jax.device_put(np.concatenate(
            [np.asarray(in_maps[c][nm]) for c in range(C)], axis=0), shd)
        for nm in in_names]
    zero_sets = [
        [jax.device_put(
            np.zeros((C * av.shape[0], *av.shape[1:]), av.dtype), shd)
         for av in out_avals]
        for _ in range(iters + 1)]
    # warm-up (compiles / binds)
    outs = sharded(*concat_in, *zero_sets[0])
    jax.block_until_ready(outs)
    t0 = time.time()
    for i in range(iters):
        outs = sharded(*concat_in, *zero_sets[1 + i])
    jax.block_until_ready(outs)
    per_call = (time.time() - t0) / iters * 1e9
    res = [
        {nm: np.asarray(outs[i]).reshape(C, *out_avals[i].shape)[c]
         for i, nm in enumerate(out_names)}
        for c in range(C)]
    return res, per_call


def kernel(x, edge_index, W1l, b1, W1r, g1, be1, rm1, rv1,
           W2l, b2, W2r, g2, be2, rm2, rv2, Wp, bp, _sim=False):
    t0 = time.time()
    x = np.asarray(x, np.float32)
    edge_index = np.asarray(edge_index)
    src = edge_index[0].astype(np.int64)
    dst = edge_index[1].astype(np.int64)

    sh, payload, pos_of_local, inv = _host_prep(src, dst)
    S = sh["S"]
    print(f"[kernel] prep done: calls L1={sh['ncalls1']} "
          f"L2={sh['ncalls2']} S={S} {time.time() - t0:.0f}s", flush=True)

    wts, wcomb = _fold_weights(W1l, b1, W1r, g1, be1, rm1, rv1,
                               W2l, b2, W2r, g2, be2, rm2, rv2, Wp, bp)
    in_maps = _make_in_maps(x, sh, payload, pos_of_local, inv, wts, wcomb)
    print(f"[kernel] inputs packed {time.time() - t0:.0f}s", flush=True)

    nc = _build(sh)
    print(f"[kernel] build+compile done {time.time() - t0:.0f}s", flush=True)

    if _sim:
        import concourse.bass_interp as bass_interp
        sim = bass_interp.MultiCoreSim(nc, C)
        for d in range(C):
            for kk, v in in_maps[d].items():
                sim.cores[d].tensor(kk)[:] = np.asarray(v).reshape(
                    sim.cores[d].tensor(kk).shape)
        sim.simulate(check_with_hw=False)
        outs = [np.asarray(sim.cores[d].mem_tensor("outd")).reshape(S)
                [pos_of_local[d]] for d in range(C)]
        return np.concatenate(outs).reshape(N, 1).astype(np.float32)

    global LAST_EXEC_NS, LAST_RESULTS
    iters = int(os.environ.get("TIMER_K", "0"))
    if iters:
        results, per_call = _timed_run(nc, in_maps, iters)
        LAST_EXEC_NS = per_call
        LAST_RESULTS = results
        outs = [np.asarray(results[d]["outd"]).reshape(S)[pos_of_local[d]]
                for d in range(C)]
        return np.concatenate(outs).reshape(N, 1).astype(np.float32)
    t1 = time.time()
    res = run_bass_kernel_spmd(nc, in_maps, core_ids=list(range(C)),
                               trace=bool(int(os.environ.get("TRACE", "0"))))
    LAST_EXEC_NS = (time.time() - t1) * 1e9
    if res.exec_time_ns:
        LAST_EXEC_NS = res.exec_time_ns
    LAST_RESULTS = res
    outs = [np.asarray(res.results[d]["outd"]).reshape(S)[pos_of_local[d]]
            for d in range(C)]
    return np.concatenate(outs).reshape(N, 1).astype(np.float32)


# revision 23
# speedup vs baseline: 1.3500x; 1.0651x over previous
"""Trainium2 Bass kernel for the 2-layer GraphSAGE bus-stop predictor.

Self-contained: kernel(**inputs) -> np.ndarray [N, 1].

Strategy (8 NeuronCores, SPMD, dst-sharded nodes):

L1 aggregation (x is only 4-wide):
  - x packed bf16, 32 nodes per 256B gather element -> the whole 1M-node
    table is 31250 elements = ONE int16 window (8MB).
  - Per-core nodes sorted into rectangular [128 x d] degree-class groups
    (d in {1,2,4,8,16,32}); gather calls stream group columns; an on-chip
    one-hot (iota==v) select + fused reduce sums each group's columns
    straight into an SBUF-resident agg1 accumulator. NO scatter-add.
L2 aggregation (h1 is 64-wide):
  - h1 stored bf16 paired (2 nodes per 256B element); chunked AllGather
    builds the global table (16 windows x 4 dst-quarters packing cells).
  - gather pair -> on-chip mask blend extracts the right half ->
    dma_scatter_add (CCE) into fp32 agg2 in HBM.
Dense phases: BN folded into weights; L1 uses a combined [x|agg] K=16
matmul; L2 keeps h1 feature-major in DRAM so the root term needs no
transpose; the 1-wide head + sigmoid is fused into the L2 loop.
"""

import os
import time

import numpy as np
import ml_dtypes

import concourse.bacc as bacc
import concourse.mybir as mybir
import concourse.tile as tile
from concourse.bass_utils import run_bass_kernel_spmd

f32 = mybir.dt.float32
bf16 = mybir.dt.bfloat16
i16 = mybir.dt.int16
BF = ml_dtypes.bfloat16

AF = mybir.ActivationFunctionType
OP = mybir.AluOpType

N = 1_000_000
C = 8                    # cores
SLOC = N // C            # nodes per core
PACK1 = 32               # x nodes per 256B gather element
XEL = N // PACK1         # 31250 L1 table elements (single int16 window)
# SWDGE call sizes. The IndirectLoad completion-semaphore wait is
# num_idxs * elem_bytes/4 (+4) and must fit 16 bits -> 1024x256B is
# exactly over. L1 keeps full 128-row columns (7 per call); L2 is flat.
CALL1 = 896
COLS1 = 7
CALL2 = 960
WIN2 = 32768             # L2 gather window (elements)
CHUNK = 512              # dense-phase nodes per chunk
CLS = (1, 2, 4, 8, 16, 32)   # padded degree classes (0 handled separately)
PADV = 200.0             # one-hot sentinel: matches no iota value
EPS = 1e-5

LAST_EXEC_NS = None
LAST_RESULTS = None


# ---------------------------------------------------------------- host prep

def _wrap_idx(vals, call):
    """int16 vals [k, call] -> dma idx tiles [k, 128, call//16]."""
    m = np.asarray(vals, dtype=np.int16).reshape(-1, call)
    k = m.shape[0]
    out = np.zeros((k, 128, call // 16), dtype=np.int16)
    ii = np.arange(call)
    for g in range(8):
        out[:, (ii % 16) + 16 * g, ii // 16] = m
    return out


def _agg2_row(pos):
    """Position -> agg2 row (in-chunk interleave so dense loads have
    1KB-contiguous runs per partition)."""
    pm = pos % CHUNK
    return (pos // CHUNK) * CHUNK + (pm % 128) * 4 + pm // 128


def _host_prep(src, dst):
    deg = np.bincount(dst, minlength=N).astype(np.int64)
    inv = (1.0 / np.maximum(deg, 1.0)).astype(np.float32)
    assert int(deg.max()) <= CLS[-1], int(deg.max())

    degc = deg.reshape(C, SLOC)
    pcls = np.zeros((C, SLOC), np.int64)
    lo = 0
    for k in CLS:
        pcls[(degc > lo) & (degc <= k)] = k
        lo = k

    cnt = {k: np.array([(pcls[c] == k).sum() for c in range(C)]) for k in CLS}
    G = {k: int(np.ceil((cnt[k].max() + 4) / 128)) for k in CLS}
    rows_used = 128 * sum(G.values())
    cnt0 = np.array([(pcls[c] == 0).sum() for c in range(C)])
    rows0 = int(np.ceil((cnt0.max() + 4) / 128)) * 128
    S = rows_used + rows0
    S = ((S + 2047) // 2048) * 2048
    q = S // 4
    assert q <= 32767, S
    reserved = np.array([q - 1, 2 * q - 1, 3 * q - 1, 4 * q - 1])
    GG = S // 128
    classbase = {}
    b = 0
    for k in CLS:
        classbase[k] = b
        b += 128 * G[k]
    zerobase = b

    # per-core incoming-edge CSR (by dst) and node positions
    pos_of_local = []
    core_csr = []
    for c in range(C):
        m = (dst // SLOC) == c
        sd = src[m]
        dl = dst[m] - c * SLOC
        o = np.argsort(dl, kind="stable")
        sd = sd[o]
        starts = np.searchsorted(dl[o], np.arange(SLOC + 1))
        core_csr.append((sd, starts))
        pos = np.full(SLOC, -1, np.int64)
        for k in CLS + (0,):
            nodes = np.nonzero(pcls[c] == k)[0]
            base = classbase[k] if k else zerobase
            length = 128 * G[k] if k else S - zerobase
            pr = base + np.arange(length)
            pr = pr[~np.isin(pr, reserved)][: len(nodes)]
            assert len(pr) == len(nodes), (c, k, len(nodes))
            pos[nodes] = pr
        pos_of_local.append(pos)

    # ---- L1 gather streams + shared reduce-fragment schedule
    frags = {}           # call -> list[(c0, c1, gg, accum)]
    callbase = {}
    ncalls1 = 0
    for k in CLS:
        callbase[k] = ncalls1
        ncalls1 += (G[k] * k + COLS1 - 1) // COLS1
        for g in range(G[k]):
            gg = classbase[k] // 128 + g
            c0 = g * k
            first = True
            while c0 < (g + 1) * k:
                call = callbase[k] + c0 // COLS1
                e = min((g + 1) * k, (c0 // COLS1 + 1) * COLS1)
                frags.setdefault(call, []).append(
                    (c0 % COLS1, ((e - 1) % COLS1) + 1, gg, not first))
                first = False
                c0 = e

    idx1 = [[] for _ in range(C)]
    vr1 = [[] for _ in range(C)]
    for c in range(C):
        sd, starts = core_csr[c]
        lens = np.diff(starts)
        pos = pos_of_local[c]
        for k in CLS:
            nodes = np.nonzero(pcls[c] == k)[0]
            rows = np.full(G[k] * 128, -1, np.int64)
            rows[pos[nodes] - classbase[k]] = nodes
            srcmat = np.full((G[k] * 128, k), -1, np.int64)
            nz = np.nonzero(rows >= 0)[0]
            nd = rows[nz]
            if len(nd):
                ln = lens[nd]
                rr = np.repeat(nz, ln)
                cc2 = (np.arange(len(rr)) -
                       np.repeat(np.cumsum(
                           np.concatenate([[0], ln[:-1]])), ln))
                flat = np.repeat(starts[nd], ln) + cc2
                srcmat[rr, cc2] = sd[flat]
            st = srcmat.reshape(G[k], 128, k).transpose(0, 2, 1).reshape(-1)
            ncall_k = (G[k] * k + COLS1 - 1) // COLS1
            padn = ncall_k * CALL1 - len(st)
            st = np.concatenate([st, np.full(padn, -1, np.int64)])
            gi = np.where(st >= 0, st // PACK1, 0).astype(np.int16)
            vv = np.where(st >= 0, st % PACK1, PADV).astype(BF)
            idx1[c].append(gi.reshape(-1, CALL1))
            vr1[c].append(vv.reshape(-1, COLS1, 128).transpose(0, 2, 1))
        idx1[c] = _wrap_idx(np.concatenate(idx1[c]), CALL1)
        vr1[c] = np.concatenate(vr1[c]).astype(BF)
    assert idx1[0].shape[0] == ncalls1

    # ---- L2 cell streams
    T2 = C * S // 2
    nwin = (T2 + WIN2 - 1) // WIN2
    cells = [dict() for _ in range(C)]
    dump_row = _agg2_row(q - 1) % q
    for c in range(C):
        sd, starts = core_csr[c]
        lens = np.diff(starts)
        pos = pos_of_local[c]
        dposn = np.repeat(pos, lens)
        sc = sd // SLOC
        agq = S // 8           # elements per core per AllGather chunk
        elem = np.empty(len(sd), np.int64)
        half = np.empty(len(sd), np.int64)
        ag_shared = bool(os.environ.get("AG_SHARED"))
        for cc in range(C):
            m2 = sc == cc
            pp = pos_of_local[cc][sd[m2] % SLOC]
            el = pp // 2
            if ag_shared:
                # single AllGather output is core-major
                elem[m2] = cc * (S // 2) + el
            else:
                # h1f is chunk-major: [chunk r][core cc][rows] so each
                # chunked AllGather output is contiguous
                elem[m2] = (el // agq) * (C * agq) + cc * agq + el % agq
            half[m2] = pp % 2
        w = elem // WIN2
        drow = _agg2_row(dposn)
        qq = drow // q
        sidx = drow % q
        order = np.lexsort((sidx, qq, w))
        elem, half, w, qq, sidx = (a[order] for a in (elem, half, w, qq, sidx))
        bounds = np.searchsorted(w * 4 + qq, np.arange(nwin * 4 + 1))
        for cell in range(nwin * 4):
            lo2, hi = bounds[cell], bounds[cell + 1]
            if lo2 == hi:
                continue
            cw, cq = cell // 4, cell % 4
            ge = elem[lo2:hi] - cw * WIN2
            hh = half[lo2:hi]
            ds = sidx[lo2:hi]
            # occurrence index within each dst run (stream is dst-sorted)
            n_e = hi - lo2
            newdst = np.ones(n_e, bool)
            newdst[1:] = ds[1:] != ds[:-1]
            runstart = np.maximum.accumulate(
                np.where(newdst, np.arange(n_e), 0))
            occ = np.arange(n_e) - runstart
            o2 = np.lexsort((ds, occ))
            ge, hh, ds = ge[o2], hh[o2], ds[o2]
            calls = []
            i = 0
            pend = []
            while i < n_e or pend:
                cg, ch, cd = [], [], []
                seen = set()
                oldp, pend = pend, []
                for p in oldp:
                    if p[2] in seen:
                        pend.append(p)
                    else:
                        cg.append(p[0])
                        ch.append(p[1])
                        cd.append(p[2])
                        seen.add(p[2])
                while i < n_e and len(cg) < CALL2:
                    if int(ds[i]) in seen:
                        pend.append((int(ge[i]), int(hh[i]), int(ds[i])))
                    else:
                        cg.append(int(ge[i]))
                        ch.append(int(hh[i]))
                        cd.append(int(ds[i]))
                        seen.add(int(ds[i]))
                    i += 1
                npad = CALL2 - len(cg)
                cg += [0] * npad
                ch += [-1] * npad
                cd += [dump_row] * npad
                calls.append((np.array(cg, np.int16), np.array(ch, np.int16),
                              np.array(cd, np.int16)))
            cells[c][(cw, cq)] = calls

    recs = []
    for cell in sorted({kk for cc in cells for kk in cc}):
        ncall = max(len(cc.get(cell, [])) for cc in cells)
        cw, cq = cell
        for kk in range(ncall):
            recs.append((kk, cw, cq))
    recs.sort()
    meta2 = []
    g2 = [[] for _ in range(C)]
    s2 = [[] for _ in range(C)]
    m2l = [[] for _ in range(C)]
    for kk, cw, cq in recs:
        meta2.append((cw * WIN2, cq))
        for c in range(C):
            cl = cells[c].get((cw, cq), [])
            if kk < len(cl):
                cg, ch, cd = cl[kk]
            else:
                cg = np.zeros(CALL2, np.int16)
                ch = np.full(CALL2, -1, np.int16)
                cd = np.full(CALL2, dump_row, np.int16)
            g2[c].append(cg)
            s2[c].append(cd)
            mm = np.zeros((1024, 2), BF)
            mm[:CALL2][ch == 0, 0] = 1
            mm[:CALL2][ch == 1, 1] = 1
            m2l[c].append(mm.reshape(8, 128, 2).transpose(1, 0, 2))
    ncalls2 = len(meta2)
    for c in range(C):
        g2[c] = _wrap_idx(np.concatenate(g2[c]), CALL2)
        s2[c] = _wrap_idx(np.concatenate(s2[c]), CALL2)
        m2l[c] = np.stack(m2l[c]).reshape(ncalls2, 128, 16)

    shared = dict(S=S, q=q, GG=GG, ncalls1=ncalls1, frags=frags,
                  meta2=meta2, ncalls2=ncalls2, T2=T2)
    payload = [dict(idx1=idx1[c], vr1=vr1[c], g2=g2[c], s2=s2[c], m2=m2l[c])
               for c in range(C)]
    return shared, payload, pos_of_local, inv


# ---------------------------------------------------------------- bass build

def _build(sh):
    S, q, GG = sh["S"], sh["q"], sh["GG"]
    ncalls1, ncalls2 = sh["ncalls1"], sh["ncalls2"]
    frags, meta2, T2 = sh["frags"], sh["meta2"], sh["T2"]
    nchunks = S // CHUNK
    NB1 = (ncalls1 + 7) // 8
    NB2 = (ncalls2 + 7) // 8

    nc = bacc.Bacc("TRN2", target_bir_lowering=False, debug=False,
                   num_devices=C, num_swdge_queues=4)
    xtab = nc.dram_tensor("xtab", [XEL, 128], bf16, kind="ExternalInput")
    iotab = nc.dram_tensor("iotab", [128, 256], bf16, kind="ExternalInput")
    idx1 = nc.dram_tensor("idx1", [128, NB1 * 8, CALL1 // 16], i16,
                          kind="ExternalInput")
    vr1 = nc.dram_tensor("vr1", [128, NB1 * 8, COLS1], bf16,
                         kind="ExternalInput")
    g2d = nc.dram_tensor("g2", [128, NB2 * 8, CALL2 // 16], i16,
                         kind="ExternalInput")
    s2d = nc.dram_tensor("s2", [128, NB2 * 8, CALL2 // 16], i16,
                         kind="ExternalInput")
    m2d = nc.dram_tensor("m2", [128, NB2 * 8, 16], bf16,
                         kind="ExternalInput")
    xr = nc.dram_tensor("xr", [128, GG, 8], f32, kind="ExternalInput")
    ivd = nc.dram_tensor("ivd", [128, GG], f32, kind="ExternalInput")
    wd = nc.dram_tensor("wd", [64, 132], f32, kind="ExternalInput")
    wcd = nc.dram_tensor("wcd", [16, 64], f32, kind="ExternalInput")
    outd = nc.dram_tensor("outd", [S], f32, kind="ExternalOutput")

    with tile.TileContext(nc) as tc:
        with tc.tile_pool(name="sb", bufs=1) as sb, \
             tc.tile_pool(name="ps", bufs=1, space="PSUM") as ps, \
             tc.tile_pool(name="dram", bufs=1, space="DRAM") as dr:

            h1b = dr.tile([S // 2, 128], bf16, tag="h1b", name="h1b")
            ag_shared = bool(os.environ.get("AG_SHARED"))
            h1f = dr.tile([C * S // 2, 128], bf16, tag="h1f", name="h1f",
                          addr_space="Shared" if ag_shared else "Local")
            hTd = dr.tile([64, S], f32, tag="hTd", name="hTd")
            agg2q = [dr.tile([q, 64], f32, tag=f"agg2q{i}",
                             name=f"agg2q{i}") for i in range(4)]

            from concourse.masks import make_identity
            ident = sb.tile([128, 128], f32, tag="ident", name="ident")
            make_identity(nc, ident[:])
            wts = sb.tile([64, 132], f32, tag="wts", name="wts")
            nc.sync.dma_start(out=wts[:], in_=wd[:])
            wcomb = sb.tile([16, 64], f32, tag="wcomb", name="wcomb")
            nc.sync.dma_start(out=wcomb[:], in_=wcd[:])
            iot = sb.tile([128, 256], bf16, tag="iot", name="iot")
            nc.sync.dma_start(out=iot[:], in_=iotab[:])
            inv_sb = sb.tile([128, GG], f32, tag="inv", name="inv_sb")
            nc.sync.dma_start(out=inv_sb[:], in_=ivd[:])

            agg1 = sb.tile([128, GG, 4], f32, tag="agg1", name="agg1")
            nc.vector.memset(agg1[:], 0.0)

            # zero agg2 (32MB) early, contiguous 8KB runs per partition
            zb = sb.tile([128, 32, 64], f32, tag="zb", name="zb")
            nc.vector.memset(zb[:], 0.0)
            zrows = 128 * 32
            for qi in range(4):
                for base in range(0, q, zrows):
                    n = min(zrows, q - base)
                    nc.sync.dma_start(
                        out=agg2q[qi][base:base + n, :]
                        .rearrange("(p t) d -> p t d", p=128),
                        in_=zb[:, :n // 128, :])

            # ---------------- L1 aggregation ----------------
            # SWDGE sem lanes are assigned round-robin in EMISSION order
            # (mod 8); queue_num must stay consistent with the lane, so
            # queue = emission index % 4 (8 % 4 == 0 keeps lane->queue 1:1)
            swop = [0]

            def nq():
                v = swop[0] % 4
                swop[0] += 1
                return v

            iov = iot[:, :COLS1 * 32].rearrange("p (c v) -> p c v", c=COLS1)
            it1 = vt1 = None
            for k in range(0 if os.environ.get("NO_L1") else ncalls1):
                if k % 8 == 0:
                    it1 = sb.tile([128, 8, CALL1 // 16], i16,
                                  tag=f"it1{(k // 8) % 2}", name=f"it1_{k}")
                    nc.sync.dma_start(out=it1[:], in_=idx1[:, k:k + 8, :])
                    vt1 = sb.tile([128, 8, COLS1], bf16,
                                  tag=f"vt1{(k // 8) % 2}", name=f"vt1_{k}")
                    nc.sync.dma_start(out=vt1[:], in_=vr1[:, k:k + 8, :])
                gb = sb.tile([128, COLS1, 128], bf16, tag=f"gb{k % 4}",
                             name=f"gb_{k}")
                nc.gpsimd.dma_gather(
                    out_ap=gb[:], in_ap=xtab[:, :],
                    idxs_ap=it1[:, k % 8, :],
                    num_idxs=CALL1, num_idxs_reg=CALL1, elem_size=128,
                    queue_num=nq())
                oh = sb.tile([128, COLS1, 32], bf16, tag=f"oh{k % 2}",
                             name=f"oh_{k}")
                nc.vector.tensor_tensor(
                    out=oh[:], in0=iov,
                    in1=vt1[:, k % 8, :].unsqueeze(-1)
                    .broadcast_to([128, COLS1, 32]),
                    op=OP.is_equal)
                tt = sb.tile([128, COLS1, 32, 4], bf16, tag=f"tt{k % 2}",
                             name=f"tt_{k}")
                nc.vector.tensor_tensor(
                    out=tt[:],
                    in0=gb[:].rearrange("p c (v f) -> p c v f", f=4),
                    in1=oh[:].unsqueeze(-1).broadcast_to(
                        [128, COLS1, 32, 4]),
                    op=OP.mult)
                for (c0, c1, gg, accum) in frags.get(k, []):
                    src_ap = tt[:, c0:c1, :, :] \
                        .rearrange("p c v f -> p f (c v)")
                    if not accum:
                        nc.vector.tensor_reduce(
                            out=agg1[:, gg, :], in_=src_ap,
                            axis=mybir.AxisListType.X, op=OP.add)
                    else:
                        tmp = sb.tile([128, 4], f32, tag="rtmp",
                                      name=f"rtmp_{k}_{gg}")
                        nc.vector.tensor_reduce(
                            out=tmp[:], in_=src_ap,
                            axis=mybir.AxisListType.X, op=OP.add)
                        nc.vector.tensor_tensor(
                            out=agg1[:, gg, :], in0=agg1[:, gg, :],
                            in1=tmp[:], op=OP.add)

            # ---------------- L1 dense (+ chunked AllGather) -------------
            comb = []
            for i in range(2):
                t = sb.tile([128, 4, 16], f32, tag=f"comb{i}",
                            name=f"comb{i}")
                nc.vector.memset(t[:], 0.0)
                comb.append(t)
            agq = S // 8            # table elements per AllGather chunk
            for ch in range(nchunks):
                g0 = ch * 4
                cb = comb[ch % 2]
                nc.sync.dma_start(out=cb[:, :, 0:8],
                                  in_=xr[:, g0:g0 + 4, :])
                nc.vector.tensor_tensor(
                    out=cb[:, :, 8:12], in0=agg1[:, g0:g0 + 4, :],
                    in1=inv_sb[:, g0:g0 + 4].unsqueeze(-1)
                    .broadcast_to([128, 4, 4]),
                    op=OP.mult)
                pT = ps.tile([16, CHUNK], f32, tag="pT", name=f"pT{ch}")
                for t in range(4):
                    nc.tensor.transpose(
                        out=pT[:, t * 128:(t + 1) * 128],
                        in_=cb[:, t, :], identity=ident[:])
                cT = sb.tile([16, CHUNK], f32, tag="cT", name=f"cT{ch}")
                nc.scalar.copy(out=cT[:], in_=pT[:])
                pm = ps.tile([64, CHUNK], f32, tag="pm", name=f"pm{ch}")
                nc.tensor.matmul(pm[:], lhsT=wcomb[:], rhs=cT[:],
                                 start=True, stop=True)
                hT = sb.tile([64, CHUNK], f32, tag="hT", name=f"hT{ch}")
                nc.scalar.activation(hT[:], pm[:], AF.Relu,
                                     bias=wts[:, 129:130], scale=1.0)
                nc.sync.dma_start(out=hTd[:, ch * CHUNK:(ch + 1) * CHUNK],
                                  in_=hT[:])
                # pack bf16 pairs: element e=ch*256+2p+j holds nodes
                # (4p+2j, 4p+2j+1) of this chunk
                pb = ps.tile([128, 2, 128], f32, tag="pb", name=f"pb{ch}")
                hTv = hT[:].rearrange("f (c s) -> f s c", s=4)
                for j in range(2):
                    for h in range(2):
                        nc.tensor.transpose(
                            out=pb[:, j, 64 * h:64 * h + 64],
                            in_=hTv[:, 2 * j + h, :],
                            identity=ident[:64, :64])
                hb = sb.tile([128, 2, 128], bf16, tag="hb", name=f"hb{ch}")
                nc.vector.tensor_copy(out=hb[:], in_=pb[:])
                nc.sync.dma_start(
                    out=h1b[ch * 256:(ch + 1) * 256, :]
                    .rearrange("(p j) v -> p j v", p=128),
                    in_=hb[:])
                if ag_shared:
                    # Shared output allows only one writer: single AllGather
                    # of the whole table, core-major (host mapping matches)
                    if ch + 1 == nchunks:
                        nc.gpsimd.collective_compute(
                            "AllGather", OP.bypass,
                            replica_groups=[list(range(C))],
                            ins=[h1b[:]],
                            outs=[h1f[:]])
                elif (ch + 1) % (nchunks // 4) == 0:
                    r = (ch + 1) // (nchunks // 4) - 1
                    if os.environ.get("NO_CC"):
                        nc.sync.dma_start(
                            out=h1f[r * C * agq:r * C * agq + agq, :],
                            in_=h1b[r * agq:(r + 1) * agq, :])
                    else:
                        nc.gpsimd.collective_compute(
                            "AllGather", OP.bypass,
                            replica_groups=[list(range(C))],
                            ins=[h1b[r * agq:(r + 1) * agq, :]],
                            outs=[h1f[r * C * agq:(r + 1) * C * agq, :]])

            # ---------------- L2 aggregation ----------------
            it2 = st2 = mt2 = None
            exts = {}
            nc2 = 0 if os.environ.get("NO_L2") else ncalls2
            for kb in range(0, nc2, 4):
                blk = range(kb, min(kb + 4, nc2))
                for k in blk:
                    if k % 8 == 0:
                        it2 = sb.tile([128, 8, CALL2 // 16], i16,
                                      tag=f"it2{(k // 8) % 2}",
                                      name=f"it2_{k}")
                        nc.sync.dma_start(out=it2[:], in_=g2d[:, k:k + 8, :])
                        st2 = sb.tile([128, 8, CALL2 // 16], i16,
                                      tag=f"st2{(k // 8) % 2}",
                                      name=f"st2_{k}")
                        nc.sync.dma_start(out=st2[:], in_=s2d[:, k:k + 8, :])
                        mt2 = sb.tile([128, 8, 16], bf16,
                                      tag=f"mt2{(k // 8) % 2}",
                                      name=f"mt2_{k}")
                        nc.sync.dma_start(out=mt2[:], in_=m2d[:, k:k + 8, :])
                    wbase, _qq = meta2[k]
                    wrows = min(WIN2, T2 - wbase)
                    gb2 = sb.tile([128, 8, 128], bf16, tag=f"gc{k % 4}",
                                  name=f"gc_{k}")
                    # slots CALL2..1023 are never written by the gather
                    nc.vector.memset(gb2[CALL2 % 128:, 7, :], 0.0)
                    nc.gpsimd.dma_gather(
                        out_ap=gb2[:], in_ap=h1f[wbase:wbase + wrows, :],
                        idxs_ap=it2[:, k % 8, :],
                        num_idxs=CALL2, num_idxs_reg=CALL2, elem_size=128,
                        queue_num=nq())
                    t2 = sb.tile([128, 8, 64, 2], bf16, tag=f"t2{k % 2}",
                                 name=f"t2_{k}")
                    nc.vector.tensor_tensor(
                        out=t2[:].rearrange("p c f two -> p c two f"),
                        in0=gb2[:].rearrange("p c (two f) -> p c two f",
                                             two=2),
                        in1=mt2[:, k % 8, :]
                        .rearrange("p (c two) -> p c two", two=2)
                        .unsqueeze(-1).broadcast_to([128, 8, 2, 64]),
                        op=OP.mult)
                    ext = sb.tile([128, 8, 64], f32, tag=f"ex{k % 4}",
                                  name=f"ex_{k}")
                    nc.vector.tensor_reduce(
                        out=ext[:], in_=t2[:], axis=mybir.AxisListType.X,
                        op=OP.add)
                    exts[k] = (ext, st2)
                if not os.environ.get("NO_SCAT"):
                    for k in blk:
                        ext, stt = exts.pop(k)
                        nc.gpsimd.dma_scatter_add(
                            out_ap=agg2q[meta2[k][1]][0:q, :],
                            in_ap=ext[:], idxs_ap=stt[:, k % 8, :],
                            num_idxs=CALL2, num_idxs_reg=CALL2, elem_size=64,
                            queue_num=nq())

            # ---------------- L2 dense + head ----------------
            for ch in range(nchunks):
                g0 = ch * 4
                a2 = sb.tile([128, 4, 64], f32, tag=f"a2{ch % 2}",
                             name=f"a2_{ch}")
                qi = (ch * CHUNK) // q
                lo = ch * CHUNK - qi * q
                nc.sync.dma_start(
                    out=a2[:],
                    in_=agg2q[qi][lo:lo + CHUNK, :]
                    .rearrange("(p t) d -> p t d", p=128))
                nc.vector.tensor_tensor(
                    out=a2[:], in0=a2[:],
                    in1=inv_sb[:, g0:g0 + 4].unsqueeze(-1)
                    .broadcast_to([128, 4, 64]),
                    op=OP.mult)
                pa = ps.tile([64, CHUNK], f32, tag="pa", name=f"pa{ch}")
                for t in range(4):
                    nc.tensor.transpose(
                        out=pa[:, t * 128:(t + 1) * 128],
                        in_=a2[:, t, :], identity=ident[:])
                aT = sb.tile([64, CHUNK], f32, tag="aT", name=f"aT{ch}")
                nc.scalar.copy(out=aT[:], in_=pa[:])
                rT = sb.tile([64, CHUNK], f32, tag=f"rT{ch % 2}",
                             name=f"rT_{ch}")
                nc.sync.dma_start(
                    out=rT[:], in_=hTd[:, ch * CHUNK:(ch + 1) * CHUNK])
                pm2 = ps.tile([64, CHUNK], f32, tag="pm2", name=f"pm2_{ch}")
                nc.tensor.matmul(pm2[:], lhsT=wts[:, 0:64], rhs=aT[:],
                                 start=True, stop=False)
                nc.tensor.matmul(pm2[:], lhsT=wts[:, 64:128], rhs=rT[:],
                                 start=False, stop=True)
                h2 = sb.tile([64, CHUNK], f32, tag="h2", name=f"h2_{ch}")
                nc.scalar.activation(h2[:], pm2[:], AF.Relu,
                                     bias=wts[:, 130:131], scale=1.0)
                po = ps.tile([1, CHUNK], f32, tag="po", name=f"po{ch}")
                nc.tensor.matmul(po[:], lhsT=wts[:, 128:129], rhs=h2[:],
                                 start=True, stop=True)
                ob = sb.tile([1, CHUNK], f32, tag="ob", name=f"ob{ch}")
                nc.scalar.activation(ob[:], po[:], AF.Sigmoid,
                                     bias=wts[0:1, 131:132], scale=1.0)
                nc.sync.dma_start(
                    out=outd[ch * CHUNK:(ch + 1) * CHUNK]
                    .rearrange("(o c) -> o c", o=1),
                    in_=ob[:])

    nc.compile()
    return nc


# ---------------------------------------------------------------- entry

def _fold_weights(W1l, b1, W1r, g1, be1, rm1, rv1,
                  W2l, b2, W2r, g2, be2, rm2, rv2, Wp, bp):
    s1 = (np.asarray(g1) / np.sqrt(np.asarray(rv1) + EPS)).astype(np.float32)
    s2 = (np.asarray(g2) / np.sqrt(np.asarray(rv2) + EPS)).astype(np.float32)
    w1l = (s1[:, None] * np.asarray(W1l)).astype(np.float32)
    w1r = (s1[:, None] * np.asarray(W1r)).astype(np.float32)
    c1 = (np.asarray(be1) + (np.asarray(b1) - np.asarray(rm1)) * s1
          ).astype(np.float32)
    w2l = (s2[:, None] * np.asarray(W2l)).astype(np.float32)
    w2r = (s2[:, None] * np.asarray(W2r)).astype(np.float32)
    c2 = (np.asarray(be2) + (np.asarray(b2) - np.asarray(rm2)) * s2
          ).astype(np.float32)
    wts = np.zeros((64, 132), np.float32)
    wts[:, 0:64] = w2l.T
    wts[:, 64:128] = w2r.T
    wts[:, 128] = np.asarray(Wp, np.float32)[0]
    wts[:, 129] = c1
    wts[:, 130] = c2
    wts[0, 131] = np.float32(np.asarray(bp).ravel()[0])
    wcomb = np.zeros((16, 64), np.float32)
    wcomb[0:4, :] = w1r.T
    wcomb[8:12, :] = w1l.T
    return wts, wcomb


def _make_in_maps(x, sh, payload, pos_of_local, inv, wts, wcomb):
    S, GG = sh["S"], sh["GG"]
    NB1 = (sh["ncalls1"] + 7) // 8
    NB2 = (sh["ncalls2"] + 7) // 8
    xtab = np.zeros((XEL * PACK1, 4), BF)
    xtab[:N] = x.astype(BF)
    xtab = xtab.reshape(XEL, 128)
    iotab = np.tile(np.arange(32, dtype=BF), (128, 8)).reshape(128, 256)

    def padk(a, nb):
        # [ncalls, 128, X] -> [128, nb*8, X]
        full = np.zeros((nb * 8,) + a.shape[1:], a.dtype)
        full[: a.shape[0]] = a
        return np.ascontiguousarray(full.transpose(1, 0, 2))

    in_maps = []
    for c in range(C):
        pos = pos_of_local[c]
        xrp = np.zeros((S, 8), np.float32)
        xrp[pos, 0:4] = x[c * SLOC:(c + 1) * SLOC]
        ivp = np.zeros(S, np.float32)
        ivp[pos] = inv[c * SLOC:(c + 1) * SLOC]
        p = payload[c]
        in_maps.append({
            "xtab": xtab,
            "iotab": iotab,
            "idx1": padk(p["idx1"], NB1),
            "vr1": padk(p["vr1"], NB1),
            "g2": padk(p["g2"], NB2),
            "s2": padk(p["s2"], NB2),
            "m2": padk(p["m2"], NB2),
            "xr": np.ascontiguousarray(
                xrp.reshape(GG, 128, 8).transpose(1, 0, 2)),
            "ivd": np.ascontiguousarray(ivp.reshape(GG, 128).T),
            "wd": wts,
            "wcd": wcomb,
        })
    return in_maps


def _timed_run(nc, in_maps, iters):
    """Replicates bass2jax.run_bass_via_pjrt with pre-transferred inputs and
    no output donation, launching `iters` back-to-back executions to
    amortize dispatch latency. Returns (per-core results, per-call ns)."""
    import jax
    import concourse.mybir as mb
    from jax.sharding import Mesh, PartitionSpec, NamedSharding
    from jax.experimental.shard_map import shard_map
    from concourse import bass2jax
    bass2jax.install_neuronx_cc_hook()

    partition_name = (nc.partition_id_tensor.name
                      if nc.partition_id_tensor else None)
    in_names, out_names, out_avals = [], [], []
    for alloc in nc.m.functions[0].allocations:
        if not isinstance(alloc, mb.MemoryLocationSet):
            continue
        name = alloc.memorylocations[0].name
        if alloc.kind == "ExternalInput":
            if name != partition_name:
                in_names.append(name)
        elif alloc.kind == "ExternalOutput":
            out_names.append(name)
            out_avals.append(jax.core.ShapedArray(
                tuple(alloc.tensor_shape), mb.dt.np(alloc.dtype)))
    n_params = len(in_names)
    all_names = in_names + out_names + (
        [partition_name] if partition_name else [])

    def _body(*args):
        operands = list(args)
        if partition_name is not None:
            operands.append(bass2jax.partition_id_tensor())
        return tuple(bass2jax._bass_exec_p.bind(
            *operands, out_avals=tuple(out_avals),
            in_names=tuple(all_names), out_names=tuple(out_names),
            lowering_input_output_aliases=(),
            sim_require_finite=True, sim_require_nnan=True, nc=nc))

    devices = jax.devices()[:C]
    mesh = Mesh(np.asarray(devices), ("core",))
    nspec = n_params + len(out_names)
    donate = tuple(range(n_params, nspec))
    sharded = jax.jit(shard_map(
        _body, mesh=mesh, in_specs=(PartitionSpec("core"),) * nspec,
        out_specs=(PartitionSpec("core"),) * len(out_names),
        check_rep=False), donate_argnums=donate, keep_unused=True)
    shd = NamedSharding(mesh, PartitionSpec("core"))
    concat_in = [
        jax.device_put(np.concatenate(
            [np.asarray(in_maps[c][nm]) for c in range(C)], axis=0), shd)
        for nm in in_names]
    zero_sets = [
        [jax.device_put(
            np.zeros((C * av.shape[0], *av.shape[1:]), av.dtype), shd)
         for av in out_avals]
        for _ in range(iters + 1)]
    # warm-up (compiles / binds)
    outs = sharded(*concat_in, *zero_sets[0])
    jax.block_until_ready(outs)
    t0 = time.time()
    for i in range(iters):
        outs = sharded(*concat_in, *zero_sets[1 + i])
    jax.block_until_ready(outs)
    per_call = (time.time() - t0) / iters * 1e9
    res = [
        {nm: np.asarray(outs[i]).reshape(C, *out_avals[i].shape)[c]
         for i, nm in enumerate(out_names)}
        for c in range(C)]
    return res, per_call


def kernel(x, edge_index, W1l, b1, W1r, g1, be1, rm1, rv1,
           W2l, b2, W2r, g2, be2, rm2, rv2, Wp, bp, _sim=False):
    t0 = time.time()
    x = np.asarray(x, np.float32)
    edge_index = np.asarray(edge_index)
    src = edge_index[0].astype(np.int64)
    dst = edge_index[1].astype(np.int64)

    sh, payload, pos_of_local, inv = _host_prep(src, dst)
    S = sh["S"]
    print(f"[kernel] prep done: calls L1={sh['ncalls1']} "
          f"L2={sh['ncalls2']} S={S} {time.time() - t0:.0f}s", flush=True)

    wts, wcomb = _fold_weights(W1l, b1, W1r, g1, be1, rm1, rv1,
                               W2l, b2, W2r, g2, be2, rm2, rv2, Wp, bp)
    in_maps = _make_in_maps(x, sh, payload, pos_of_local, inv, wts, wcomb)
    print(f"[kernel] inputs packed {time.time() - t0:.0f}s", flush=True)

    nc = _build(sh)
    print(f"[kernel] build+compile done {time.time() - t0:.0f}s", flush=True)

    if _sim:
        import concourse.bass_interp as bass_interp
        sim = bass_interp.MultiCoreSim(nc, C)
        for d in range(C):
            for kk, v in in_maps[d].items():
                sim.cores[d].tensor(kk)[:] = np.asarray(v).reshape(
                    sim.cores[d].tensor(kk).shape)
        sim.simulate(check_with_hw=False)
        outs = [np.asarray(sim.cores[d].mem_tensor("outd")).reshape(S)
                [pos_of_local[d]] for d in range(C)]
        return np.concatenate(outs).reshape(N, 1).astype(np.float32)

    global LAST_EXEC_NS, LAST_RESULTS
    iters = int(os.environ.get("TIMER_K", "0"))
    if iters:
        results, per_call = _timed_run(nc, in_maps, iters)
        LAST_EXEC_NS = per_call
        LAST_RESULTS = results
        outs = [np.asarray(results[d]["outd"]).reshape(S)[pos_of_local[d]]
                for d in range(C)]
        return np.concatenate(outs).reshape(N, 1).astype(np.float32)
    t1 = time.time()
    res = run_bass_kernel_spmd(nc, in_maps, core_ids=list(range(C)),
                               trace=bool(int(os.environ.get("TRACE", "0"))))
    LAST_EXEC_NS = (time.time() - t1) * 1e9
    if res.exec_time_ns:
        LAST_EXEC_NS = res.exec_time_ns
    LAST_RESULTS = res
    outs = [np.asarray(res.results[d]["outd"]).reshape(S)[pos_of_local[d]]
            for d in range(C)]
    return np.concatenate(outs).reshape(N, 1).astype(np.float32)
